# revision 1
# baseline (speedup 1.0000x reference)
"""Trainium2 Bass kernel for nn_Decoder (LSTM decoder + dual attention).

Sharding: data-parallel over batch B=128 across 8 NeuronCores (16 samples each).
On-chip layouts are feature-major ("transposed": features on partitions, time x
batch on the free dim) so biases are per-partition activation biases, the LSTM
emits gates directly in feature-major layout, and attention/projections run as
time-batched GEMMs with stationary weights.

Phases per core:
  P1: K/V projections (char+tag), X@Wih^T for all steps -> DRAM scratch.
  P2: sequential LSTM over T steps: 64 self-loading matmuls/step (Whh^T tiles
      stationary, h streaming), elementwise cell update staggered per E-chunk
      so the next step's matmuls overlap the current step's elementwise tail.
  P3: softmax attention + output projections as 128-step-block GEMMs.

Weights arrive pre-transposed / bf16 from the host (layout prep only); matmul
accumulation is fp32 in PSUM and the LSTM cell state stays fp32.
"""

import contextlib

import numpy as np
import ml_dtypes

B, T, E, G, NCH, SC, STG = 128, 256, 512, 2048, 128, 256, 32
NCORES = 8
PB = B // NCORES  # per-core batch = 16
EC = E // 128     # E chunks = 4

_cache = {}


def _build(Ts, reps=1):
    import concourse.mybir as mybir
    from concourse import bacc
    from concourse import masks
    from concourse.tile import TileContext

    dt = mybir.dt
    AF = mybir.ActivationFunctionType
    AX = mybir.AxisListType
    TB = min(128, Ts)            # P3 time-block size
    NBLK = Ts // TB
    SLAB = min(8, Ts)            # P2 xwt prefetch slab (steps)
    QE = float(1.0 / np.sqrt(E))

    nc = bacc.Bacc(None, dynamic_dma_scratch_size=4096)

    def din(name, shape, d=dt.bfloat16):
        return nc.dram_tensor(name, shape, d, kind="ExternalInput")

    ceT_d = din("ceT", [PB, E, SC])
    teT_d = din("teT", [PB, E, STG])
    xT_d = din("xT", [NCH, Ts, PB])
    whhT_d = din("whhT", [E, G])
    wihT_d = din("wihT", [NCH, G])
    PNAMES = ("wqcT", "wocT", "wqtT", "wotT")
    KNAMES = ("wkcT", "wvcT", "wktT", "wvtT")
    w_d = {nm: din(nm, [E, E]) for nm in PNAMES + KNAMES}
    outWT_d = din("outWT", [2 * E, NCH])
    gbias_d = din("gbias", [G], dt.float32)
    bias_d = {nm: din(nm, [E], dt.float32)
              for nm in ("bqc", "bvc", "boc", "bqt", "bvt", "bot")}
    outb_d = din("outb", [NCH], dt.float32)
    h0T_d = din("h0T", [E, PB])
    c0T_d = din("c0T", [E, PB], dt.float32)

    xwt_d = nc.dram_tensor("xwt", [Ts, EC, 4, 128, PB], dt.bfloat16)
    out_d = nc.dram_tensor("out", [PB, Ts, NCH], dt.float32, kind="ExternalOutput")

    with TileContext(nc) as tc, contextlib.ExitStack() as ctx:
        pp = ctx.enter_context(tc.tile_pool(name="persist", bufs=1))
        rep_cm = tc.For_i(0, reps, 1) if reps > 1 else None

        # ---- persistent tiles -------------------------------------------
        hT = pp.tile([128, EC, Ts, PB], dt.bfloat16)      # h after each step
        cT = pp.tile([128, EC, PB], dt.float32)
        h0T = pp.tile([128, EC, PB], dt.bfloat16)
        kcT = pp.tile([128, EC, PB, SC], dt.bfloat16)     # K_char^T per sample
        vc = pp.tile([128, 2, PB, E], dt.bfloat16)        # V_char [s,e] per sample
        ktT = pp.tile([128, EC, PB, STG], dt.bfloat16)
        vt = pp.tile([STG, PB, E], dt.bfloat16)           # V_tag, partitions 0..31
        wsb = {nm: pp.tile([128, EC, EC, 128], dt.bfloat16, name=nm)
               for nm in PNAMES}
        outWT = pp.tile([128, 2 * EC, NCH], dt.bfloat16)
        gbias = pp.tile([128, 16], dt.float32)
        bsb = {nm: pp.tile([128, EC], dt.float32, name=nm) for nm in bias_d}
        outb = pp.tile([128, 1], dt.float32)
        id_bf = pp.tile([128, 128], dt.bfloat16)
        id_f32 = pp.tile([128, 128], dt.float32)
        masks.make_identity(nc, id_bf[:, :])
        masks.make_identity(nc, id_f32[:, :])

        nc.sync.dma_start(h0T[:, :, :], h0T_d.rearrange("(k p) b -> p k b", p=128))
        nc.sync.dma_start(cT[:, :, :], c0T_d.rearrange("(k p) b -> p k b", p=128))
        nc.sync.dma_start(gbias[:, :], gbias_d.rearrange("(j p) -> p j", p=128))
        for nm in bias_d:
            nc.sync.dma_start(bsb[nm][:, :],
                              bias_d[nm].rearrange("(k p) -> p k", p=128))
        nc.sync.dma_start(outb[:, :], outb_d[:, None])
        for nm in PNAMES:
            for k in range(EC):
                nc.sync.dma_start(
                    wsb[nm][:, k, :, :],
                    w_d[nm][k * 128:(k + 1) * 128, :]
                    .rearrange("p (m c) -> p m c", c=128),
                )
        nc.sync.dma_start(outWT[:, :, :],
                          outWT_d.rearrange("(k p) n -> p k n", p=128))

        # =================================================================
        # P1: XWT GEMM -> DRAM scratch; K/V projections
        # =================================================================
        if rep_cm is not None:
            ctx.enter_context(rep_cm)
            # each repetition restarts from the initial cell state
            nc.sync.dma_start(cT[:, :, :],
                              c0T_d.rearrange("(k p) b -> p k b", p=128))
        with tc.tile_pool(name="p1", bufs=1) as p1, \
             tc.tile_pool(name="p1w", bufs=2) as p1w, \
             tc.tile_pool(name="ps1", bufs=3, space="PSUM") as ps1:
            wih = p1.tile([128, 16, 128], dt.bfloat16)
            nc.sync.dma_start(wih[:, :, :],
                              wihT_d.rearrange("p (j c) -> p j c", c=128))
            wkv = {nm: p1.tile([128, EC, EC, 128], dt.bfloat16, name=nm)
                   for nm in KNAMES}
            for nm in KNAMES:
                for k in range(EC):
                    nc.sync.dma_start(
                        wkv[nm][:, k, :, :],
                        w_d[nm][k * 128:(k + 1) * 128, :]
                        .rearrange("p (m c) -> p m c", c=128),
                    )
            ceT = p1.tile([128, EC, PB, SC], dt.bfloat16)
            teT = p1.tile([128, EC, PB, STG], dt.bfloat16)
            for k in range(EC):
                nc.sync.dma_start(
                    ceT[:, k, :, :],
                    ceT_d[:, k * 128:(k + 1) * 128, :].rearrange("i p s -> p i s"),
                )
                nc.sync.dma_start(
                    teT[:, k, :, :],
                    teT_d[:, k * 128:(k + 1) * 128, :].rearrange("i p s -> p i s"),
                )

            # --- XWT = Wih^T @ x, bias folded, spilled to DRAM ---
            ncc = max(1, Ts * PB // 512)
            tpc = Ts // ncc
            for cc in range(ncc):
                xbuf = p1w.tile([128, tpc, PB], dt.bfloat16, tag="xbuf")
                nc.sync.dma_start(xbuf[:, :, :],
                                  xT_d[:, cc * tpc:(cc + 1) * tpc, :])
                for j in range(16):
                    ps = ps1.tile([128, tpc, PB], dt.float32, tag="ps")
                    nc.tensor.matmul(ps[:, :, :], wih[:, j, :], xbuf[:, :, :])
                    stg = p1w.tile([128, tpc, PB], dt.bfloat16, tag="stg",
                                   bufs=4)
                    if j % 2 == 0:
                        nc.vector.tensor_scalar_add(stg[:, :, :], ps[:, :, :],
                                                    gbias[:, j:j + 1])
                    else:
                        nc.scalar.activation(stg[:, :, :], ps[:, :, :],
                                             AF.Identity,
                                             bias=gbias[:, j:j + 1])
                    nc.sync.dma_start(
                        xwt_d[cc * tpc:(cc + 1) * tpc, j % 4, j // 4, :, :]
                        .rearrange("t p b -> p t b"),
                        stg[:, :, :],
                    )

            # --- kcT[m, i, s] ---
            for m in range(EC):
                for i2 in range(0, PB, 2):
                    ps = ps1.tile([128, 2, SC], dt.float32, tag="ps")
                    for k in range(EC):
                        nc.tensor.matmul(
                            ps[:, :, :], wkv["wkcT"][:, k, m, :],
                            ceT[:, k, i2:i2 + 2, :],
                            start=(k == 0), stop=(k == EC - 1),
                        )
                    nc.any.tensor_copy(kcT[:, m, i2:i2 + 2, :], ps[:, :, :])
            # --- vc[sc, i, e] ---
            for i in range(PB):
                for sc in range(SC // 128):
                    ps = ps1.tile([128, E], dt.float32, tag="ps")
                    for k in range(EC):
                        nc.tensor.matmul(
                            ps[:, :], ceT[:, k, i, sc * 128:(sc + 1) * 128],
                            wkv["wvcT"][:, k, :, :].rearrange("p m c -> p (m c)"),
                            start=(k == 0), stop=(k == EC - 1),
                        )
                    nc.any.tensor_copy(vc[:, sc, i, :], ps[:, :])
            # --- ktT[m, i, s] ---
            for m in range(EC):
                ps = ps1.tile([128, PB, STG], dt.float32, tag="ps")
                for k in range(EC):
                    nc.tensor.matmul(
                        ps[:, :, :], wkv["wktT"][:, k, m, :], teT[:, k, :, :],
                        start=(k == 0), stop=(k == EC - 1),
                    )
                nc.any.tensor_copy(ktT[:, m, :, :], ps[:, :, :])
            # --- vt[s, i, e] (partitions 0..31) ---
            for i in range(PB):
                ps = ps1.tile([STG, E], dt.float32, tag="ps")
                for k in range(EC):
                    nc.tensor.matmul(
                        ps[:, :], teT[:, k, i, :],
                        wkv["wvtT"][:, k, :, :].rearrange("p m c -> p (m c)"),
                        start=(k == 0), stop=(k == EC - 1),
                    )
                nc.any.tensor_copy(vt[:, i, :], ps[:, :])

        # =================================================================
        # P2: sequential LSTM.  gate-tile j = gt*4 + ec, gt in (i,f,g,o)
        # =================================================================
        with tc.tile_pool(name="p2", bufs=1) as p2p, \
             tc.tile_pool(name="p2w", bufs=3) as p2, \
             tc.tile_pool(name="p2s", bufs=2) as p2s, \
             contextlib.ExitStack() as ctx2:
            gps = [ctx2.enter_context(
                tc.tile_pool(name=f"g{e}", bufs=1, space="PSUM"))
                for e in range(EC)]
            whh = p2p.tile([128, EC, 16, 128], dt.bfloat16)
            for k in range(EC):
                nc.sync.dma_start(
                    whh[:, k, :, :],
                    whhT_d[k * 128:(k + 1) * 128, :]
                    .rearrange("p (j c) -> p j c", c=128),
                )
            for t in range(Ts):
                if t % SLAB == 0:
                    slab = p2s.tile([128, SLAB, EC, 4, PB], dt.bfloat16, tag="slab")
                    for e in range(EC):
                        for g in range(4):
                            nc.sync.dma_start(
                                slab[:, :, e, g, :],
                                xwt_d[t:t + SLAB, e, g, :, :]
                                .rearrange("t p b -> p t b"),
                            )
                # gate order is (i, f, o, g) after the host-side permute, so
                # one sigmoid covers i|f|o of all E-chunks and one tanh all g
                ga = p2.tile([128, EC, 4, PB], dt.float32, tag="ga")
                for ec in range(EC):
                    gp = gps[ec].tile([128, 4, PB], dt.float32, tag=f"gt{ec}")
                    for k in range(EC):
                        rhs = h0T[:, k, :] if t == 0 else hT[:, k, t - 1, :]
                        for gt in range(4):
                            nc.tensor.matmul(
                                gp[:, gt, :], whh[:, k, gt * 4 + ec, :], rhs,
                                start=(k == 0 and gt == 0),
                                stop=(k == EC - 1 and gt == 3),
                            )
                    nc.vector.tensor_add(ga[:, ec, :, :], gp[:, :, :],
                                         slab[:, t % SLAB, ec, :, :])
                sio = p2.tile([128, EC, 3, PB], dt.float32, tag="sio")
                tg = p2.tile([128, EC, PB], dt.float32, tag="tg")
                nc.scalar.activation(sio[:, :, :, :], ga[:, :, 0:3, :], AF.Sigmoid)
                nc.scalar.activation(tg[:, :, :], ga[:, :, 3, :], AF.Tanh)
                v = p2.tile([128, EC, PB], dt.float32, tag="v")
                u = p2.tile([128, EC, PB], dt.float32, tag="u")
                nc.vector.tensor_mul(v[:, :, :], sio[:, :, 1, :], cT[:, :, :])
                nc.vector.tensor_mul(u[:, :, :], sio[:, :, 0, :], tg[:, :, :])
                nc.vector.tensor_add(cT[:, :, :], u[:, :, :], v[:, :, :])
                tcc = p2.tile([128, EC, PB], dt.float32, tag="tcc")
                nc.scalar.activation(tcc[:, :, :], cT[:, :, :], AF.Tanh)
                nc.vector.tensor_mul(hT[:, :, t, :], sio[:, :, 2, :], tcc[:, :, :])

        # =================================================================
        # P3: attention + projections, per time block
        # =================================================================
        with tc.tile_pool(name="p3", bufs=1) as p3, \
             tc.tile_pool(name="p3w", bufs=2) as p3w, \
             tc.tile_pool(name="ps3", bufs=6, space="PSUM") as ps3:
            ncols = TB * PB
            nch = max(1, ncols // 512)
            cw = ncols // nch

            def proj(dst, wname, t0, bias, scale):
                # dst[:, m, trange, :] = scale*(W^T @ hT-block) + bias
                tw = cw // PB
                for m in range(EC):
                    for cc in range(nch):
                        ps = ps3.tile([128, tw, PB], dt.float32, tag="ps")
                        for k in range(EC):
                            nc.tensor.matmul(
                                ps[:, :, :], wsb[wname][:, k, m, :],
                                hT[:, k, t0 + cc * tw:t0 + (cc + 1) * tw, :],
                                start=(k == 0), stop=(k == EC - 1),
                            )
                        nc.vector.tensor_scalar(
                            dst[:, m, cc * tw:(cc + 1) * tw, :], ps[:, :, :],
                            scale, bias[:, m:m + 1],
                            op0=mybir.AluOpType.mult, op1=mybir.AluOpType.add,
                        )

            def oproj(dst, wname, src, bias):
                # dst[:, m, (i t)] = relu(W^T @ src + bias)
                for m in range(EC):
                    for cc in range(nch):
                        ps = ps3.tile([128, cw], dt.float32, tag="ps")
                        for k in range(EC):
                            nc.tensor.matmul(
                                ps[:, :], wsb[wname][:, k, m, :],
                                src[:, k, :, :].rearrange("p i t -> p (i t)")
                                [:, cc * cw:(cc + 1) * cw],
                                start=(k == 0), stop=(k == EC - 1),
                            )
                        nc.vector.tensor_scalar(
                            dst[:, m, :, :].rearrange("p i t -> p (i t)")
                            [:, cc * cw:(cc + 1) * cw],
                            ps[:, :], bias[:, m:m + 1], 0.0,
                            op0=mybir.AluOpType.add, op1=mybir.AluOpType.max,
                        )

            for blk in range(NBLK):
                t0 = blk * TB
                # ---- char attention ----
                qT = p3.tile([128, EC, TB, PB], dt.bfloat16, tag="qT")
                proj(qT, "wqcT", t0, bsb["bqc"], QE)
                ctxT = p3.tile([128, EC, PB, TB], dt.bfloat16, tag="ctxT")
                for i in range(PB):
                    pc = ps3.tile([128, SC], dt.float32, tag="ps")
                    for k in range(EC):
                        nc.tensor.matmul(
                            pc[:TB, :], qT[:, k, :, i], kcT[:, k, i, :],
                            start=(k == 0), stop=(k == EC - 1),
                        )
                    pe = p3w.tile([128, SC], dt.bfloat16, tag="pe", bufs=1)
                    dsum = p3w.tile([128, 1], dt.float32, tag="dsum")
                    nc.scalar.activation(pe[:TB, :], pc[:TB, :], AF.Exp,
                                         accum_out=dsum[:TB, :])
                    drec = p3w.tile([128, 1], dt.float32, tag="drec")
                    nc.vector.reciprocal(drec[:TB, :], dsum[:TB, :])
                    pn = p3w.tile([128, SC], dt.bfloat16, tag="pn", bufs=1)
                    nc.vector.tensor_scalar_mul(pn[:TB, :], pe[:TB, :],
                                                drec[:TB, 0:1])
                    pTt = p3w.tile([128, 2, 128], dt.bfloat16, tag="pTt")
                    for sc in range(2):
                        tp = ps3.tile([128, 128], dt.bfloat16, tag="ps")
                        nc.tensor.transpose(
                            tp[:, :TB], pn[:TB, sc * 128:(sc + 1) * 128],
                            id_bf[:TB, :TB],
                        )
                        nc.vector.tensor_copy(pTt[:, sc, :TB], tp[:, :TB])
                    cps = ps3.tile([128, EC, 128], dt.float32, tag="ps")
                    for m in range(EC):
                        for sc in range(2):
                            nc.tensor.matmul(
                                cps[:, m, :TB],
                                vc[:, sc, i, m * 128:(m + 1) * 128],
                                pTt[:, sc, :TB],
                                start=(m == 0 and sc == 0),
                                stop=(m == EC - 1 and sc == 1),
                            )
                    for m in range(EC):
                        nc.vector.tensor_scalar_add(
                            ctxT[:, m, i, :], cps[:, m, :TB],
                            bsb["bvc"][:, m:m + 1],
                        )
                orc = p3.tile([128, EC, PB, TB], dt.bfloat16, tag="orc")
                oproj(orc, "wocT", ctxT, bsb["boc"])

                # ---- tag attention ----
                qT2 = p3.tile([128, EC, TB, PB], dt.bfloat16, tag="qT")
                proj(qT2, "wqtT", t0, bsb["bqt"], QE)
                ptp = ps3.tile([128, PB, STG], dt.float32, tag="ps")
                for i in range(PB):
                    for k in range(EC):
                        nc.tensor.matmul(
                            ptp[:TB, i, :], qT2[:, k, :, i], ktT[:, k, i, :],
                            start=(i == 0 and k == 0),
                            stop=(i == PB - 1 and k == EC - 1),
                        )
                pte = p3w.tile([128, PB, STG], dt.bfloat16, tag="pte", bufs=1)
                nc.scalar.activation(pte[:TB, :, :], ptp[:TB, :, :], AF.Exp)
                tsum = p3w.tile([128, PB], dt.float32, tag="tsum")
                nc.vector.reduce_sum(tsum[:TB, :], pte[:TB, :, :], axis=AX.X)
                trec = p3w.tile([128, PB], dt.float32, tag="trec")
                nc.vector.reciprocal(trec[:TB, :], tsum[:TB, :])
                ptn = p3w.tile([128, PB, STG], dt.bfloat16, tag="ptn", bufs=1)
                ptT = p3w.tile([STG, PB, TB], dt.bfloat16, tag="ptT", bufs=1)
                for i in range(PB):
                    nc.vector.tensor_scalar_mul(ptn[:TB, i, :], pte[:TB, i, :],
                                                trec[:TB, i:i + 1])
                    tp2 = ps3.tile([STG, 128], dt.bfloat16, tag="ps")
                    nc.tensor.transpose(tp2[:, :TB], ptn[:TB, i, :],
                                        id_bf[:TB, :TB])
                    nc.vector.tensor_copy(ptT[:, i, :], tp2[:, :TB])
                ctxT2 = p3.tile([128, EC, PB, TB], dt.bfloat16, tag="ctxT")
                for i in range(PB):
                    cps = ps3.tile([128, EC, 128], dt.float32, tag="ps")
                    for m in range(EC):
                        nc.tensor.matmul(
                            cps[:, m, :TB], vt[:, i, m * 128:(m + 1) * 128],
                            ptT[:, i, :],
                            start=(m == 0), stop=(m == EC - 1),
                        )
                    for m in range(EC):
                        nc.vector.tensor_scalar_add(
                            ctxT2[:, m, i, :], cps[:, m, :TB],
                            bsb["bvt"][:, m:m + 1],
                        )
                ort = p3.tile([128, EC, PB, TB], dt.bfloat16, tag="ort")
                oproj(ort, "wotT", ctxT2, bsb["bot"])

                # ---- output projection + per-sample transpose ----
                for cc in range(nch):
                    ps = ps3.tile([128, cw], dt.float32, tag="ps")
                    for k in range(2 * EC):
                        src = orc if k < EC else ort
                        nc.tensor.matmul(
                            ps[:, :], outWT[:, k, :],
                            src[:, k % EC, :, :].rearrange("p i t -> p (i t)")
                            [:, cc * cw:(cc + 1) * cw],
                            start=(k == 0), stop=(k == 2 * EC - 1),
                        )
                    of = p3w.tile([128, cw], dt.float32, tag="of")
                    nc.vector.tensor_scalar_add(of[:, :], ps[:, :],
                                                outb[:, 0:1])
                    ns = cw // TB
                    for si in range(ns):
                        i = cc * ns + si
                        tps = ps3.tile([128, 128], dt.float32, tag="ps")
                        nc.tensor.transpose(
                            tps[:TB, :], of[:, si * TB:(si + 1) * TB],
                            id_f32[:, :],
                        )
                        oseg = p3w.tile([TB, NCH], dt.float32, tag="oseg")
                        nc.vector.tensor_copy(oseg[:, :], tps[:TB, :])
                        nc.sync.dma_start(out_d[i, t0:t0 + TB, :], oseg[:, :])

    nc.compile()
    return nc


# gate-row permutation: torch order (i,f,g,o) -> kernel order (i,f,o,g)
_GPERM = np.r_[0:E, E:2 * E, 3 * E:4 * E, 2 * E:3 * E]


def _prep_core(inputs, core, Ts=T):
    bf = ml_dtypes.bfloat16
    s = slice(core * PB, (core + 1) * PB)
    ce = inputs["char_encoding"][s]
    te = inputs["tag_encoding"][s]
    tos = inputs["true_output_seq"][s][:, :Ts]
    xs = np.concatenate(
        [np.zeros((PB, 1, NCH), np.float32), tos[:, 1:, :]], axis=1
    )
    m = {
        "ceT": np.ascontiguousarray(ce.transpose(0, 2, 1)).astype(bf),
        "teT": np.ascontiguousarray(te.transpose(0, 2, 1)).astype(bf),
        "xT": np.ascontiguousarray(xs.transpose(2, 1, 0)).astype(bf),
        "whhT": np.ascontiguousarray(inputs["lstm_Whh"][_GPERM].T).astype(bf),
        "wihT": np.ascontiguousarray(inputs["lstm_Wih"][_GPERM].T).astype(bf),
        "wqcT": np.ascontiguousarray(inputs["ca_Wq"].T).astype(bf),
        "wkcT": np.ascontiguousarray(inputs["ca_Wk"].T).astype(bf),
        "wvcT": np.ascontiguousarray(inputs["ca_Wv"].T).astype(bf),
        "wocT": np.ascontiguousarray(inputs["ca_Wo"].T).astype(bf),
        "wqtT": np.ascontiguousarray(inputs["ta_Wq"].T).astype(bf),
        "wktT": np.ascontiguousarray(inputs["ta_Wk"].T).astype(bf),
        "wvtT": np.ascontiguousarray(inputs["ta_Wv"].T).astype(bf),
        "wotT": np.ascontiguousarray(inputs["ta_Wo"].T).astype(bf),
        "outWT": np.ascontiguousarray(inputs["out_W"].T).astype(bf),
        "gbias": (inputs["lstm_bih"] + inputs["lstm_bhh"])[_GPERM]
        .astype(np.float32),
        "bqc": (inputs["ca_bq"] / np.sqrt(E)).astype(np.float32),
        "bvc": inputs["ca_bv"].astype(np.float32),
        "boc": inputs["ca_bo"].astype(np.float32),
        "bqt": (inputs["ta_bq"] / np.sqrt(E)).astype(np.float32),
        "bvt": inputs["ta_bv"].astype(np.float32),
        "bot": inputs["ta_bo"].astype(np.float32),
        "outb": inputs["out_b"].astype(np.float32),
        "h0T": np.ascontiguousarray(
            np.concatenate([inputs["char_hn"][0][s],
                            inputs["char_hn"][1][s]], -1).T).astype(bf),
        "c0T": np.ascontiguousarray(
            np.concatenate([inputs["char_cn"][0][s],
                            inputs["char_cn"][1][s]], -1).T).astype(np.float32),
    }
    return m


def kernel(**inputs):
    from concourse.bass_utils import run_bass_kernel_spmd

    inputs = {k: np.asarray(v, dtype=np.float32) for k, v in inputs.items()}
    if "nc" not in _cache:
        _cache["nc"] = _build(T)
    nc = _cache["nc"]
    in_maps = [_prep_core(inputs, c) for c in range(NCORES)]
    res = run_bass_kernel_spmd(nc, in_maps, list(range(NCORES)))
    _cache["last_res"] = res
    outs = [np.asarray(res.results[c]["out"]) for c in range(NCORES)]
    return np.concatenate(outs, axis=0).astype(np.float32)



# revision 25
# speedup vs baseline: 1.4159x; 1.4159x over previous
"""Trainium2 Bass kernel for nn_Decoder (LSTM decoder + dual attention).

Sharding: data-parallel over batch B=128 across 8 NeuronCores (16 samples each).
Feature-major on-chip layouts (features on partitions, time x batch on the free
dim).  Key optimizations over the naive structure:

  - xwt (X @ Wih^T) DRAM scratch stored gate-tile-major so both the P1 write
    and the P2 per-step read use >=512B contiguous runs (the naive layout
    produced 32B scatter descriptors and ran at ~4.5 GB/s).
  - Whh stored fp8e4m3: LDWEIGHTS with FWL runs ~2x faster than bf16, and the
    LSTM recurrence is weight-load-bound (64 self-loading 128x128 tiles per
    step, only 16 streaming columns each).  h stays bf16 (mixed-dtype matmul).
  - P2 emission order: per step, gate tiles are computed half-by-half
    (features 0:256 then 256:512) with the cell update of each half emitted
    immediately after its matmuls, so the elementwise tail of step t overlaps
    the leading matmuls of step t+1.
  - Attention algebra: softmax drops per-query constants, so
    scores = (h @ (Wq^T Wk) / sqrt(E) + bq Wk / sqrt(E)) @ ce^T  -- the K
    projection disappears.  Wo folds into V (V' = (ce Wv^T + bv) Wo^T), so the
    output projection of each attention disappears.  For the tag attention the
    fold goes into the encoding side (te~ = te M^T), which is 8x smaller than
    projecting all queries.
  - Probability transposes via DMA xbar transpose (no PE/PSUM round trip).
  - Output written feature-major ([B, NCH, T]) with 512B runs; the host does
    the final cheap transpose to [B, T, NCH].
"""

import contextlib

import numpy as np
import ml_dtypes

B, T, E, G, NCH, SC, STG = 128, 256, 512, 2048, 128, 256, 32
NCORES = 8
PB = B // NCORES  # per-core batch = 16
EC = E // 128     # E chunks = 4
SLAB = 16         # P2 xwt prefetch window (steps)

_cache = {}


def _build(Ts):
    import concourse.mybir as mybir
    from concourse import bacc
    from concourse.tile import TileContext

    dt = mybir.dt
    AF = mybir.ActivationFunctionType
    AX = mybir.AxisListType
    AL = mybir.AluOpType
    TB = min(128, Ts)            # P3 time-block size
    NBLK = Ts // TB
    QE = float(1.0 / np.sqrt(E))

    nc = bacc.Bacc(None, dynamic_dma_scratch_size=4096)

    def din(name, shape, d=dt.bfloat16):
        return nc.dram_tensor(name, shape, d, kind="ExternalInput")

    ceT_d = din("ceT", [PB, E, SC])
    teT_d = din("teT", [PB, E, STG])
    xT_d = din("xT", [NCH, Ts, PB])
    whh_d = din("whhP", [E, 16, 128], dt.float8e3)   # 16*Whh, e3m4
    whhb_d = din("whhB", [E, 16, 128])               # Whh bf16 (step-0 gates)
    wih_d = din("wihP", [NCH, 16, 128])
    gbias_d = din("gbias", [16, 128], dt.float32)
    # raw (untransposed, torch [out,in]) projection weights for on-device folds
    wqc_d = din("wqc", [E, E])
    wkc_d = din("wkc", [E, E])
    wvc_d = din("wvc", [E, E])
    wocT_d = din("wocT", [E, E])
    wqt_d = din("wqt", [E, E])
    wkt_d = din("wkt", [E, E])
    wvt_d = din("wvt", [E, E])
    wotT_d = din("wotT", [E, E])
    bqc_d = din("bqc_col", [E, 1])
    bvc_d = din("bvc_col", [E, 1])
    bqt_d = din("bqt_col", [E, 1])
    bvt_d = din("bvt_col", [E, 1])
    boc_d = din("boc", [E], dt.float32)
    bot_d = din("bot", [E], dt.float32)
    outWT_d = din("outWT", [2 * E, NCH])
    outb_d = din("outb", [NCH], dt.float32)
    h0T_d = din("h0T", [E, PB])
    c0T_d = din("c0T", [E, PB], dt.float32)

    xwt_d = nc.dram_tensor("xwt", [16, 128, Ts, PB], dt.bfloat16)
    out_d = nc.dram_tensor("out", [PB, NCH, Ts], dt.float32, kind="ExternalOutput")
    hdbg_d = None
    if _cache.get("debug_h"):
        hdbg_d = nc.dram_tensor("hdbg", [128, EC, Ts, PB], dt.float32,
                                kind="ExternalOutput")
    p2dbg = {}
    if _cache.get("debug_p2"):
        p2dbg["g0"] = nc.dram_tensor("g0dbg", [128, 2, 4, 2, PB], dt.float32,
                                     kind="ExternalOutput")
        p2dbg["sl"] = nc.dram_tensor("sldbg", [128, 2, 4, 2, PB], dt.bfloat16,
                                     kind="ExternalOutput")
        p2dbg["ga"] = nc.dram_tensor("gadbg", [128, 2, 4, 2, PB], dt.float32,
                                     kind="ExternalOutput")
        p2dbg["c1"] = nc.dram_tensor("c1dbg", [128, EC, PB], dt.float32,
                                     kind="ExternalOutput")

    with TileContext(nc) as tc, contextlib.ExitStack() as ctx:
        pp = ctx.enter_context(tc.tile_pool(name="persist", bufs=1))

        # ---- persistent tiles -------------------------------------------
        hT = pp.tile([128, EC, Ts, PB], dt.bfloat16)
        cT = pp.tile([128, EC, PB], dt.float32)
        h0 = pp.tile([128, EC, PB], dt.bfloat16)
        ce = pp.tile([128, EC, PB, SC], dt.bfloat16)
        te = pp.tile([128, EC, PB, STG], dt.bfloat16)
        whh = pp.tile([128, EC, 16, 128], dt.float8e3)
        g0 = pp.tile([128, 2, 4, 2, PB], dt.float32)   # Whh @ h0 (bf16, P1)
        gb = pp.tile([128, 16], dt.float32)
        mc = pp.tile([128, EC, EC, 128], dt.bfloat16)      # M_c = Wq^T Wk tiles
        bqe = pp.tile([128, EC], dt.float32)               # v_c * QE
        vcp = pp.tile([128, 2, PB, E], dt.bfloat16)        # V'_char [s, i, g]
        vtp = pp.tile([128, PB, E], dt.bfloat16)           # V'_tag, 4x replicated
        te2 = pp.tile([128, EC, PB, STG], dt.bfloat16)     # te~^T [e, i, s]
        beta = pp.tile([1, PB * STG], dt.bfloat16)
        ones1 = pp.tile([1, 128], dt.bfloat16)
        badc = pp.tile([128, EC], dt.float32)              # bvo_c + bo_c
        badt = pp.tile([128, EC], dt.float32)
        oW = pp.tile([128, 2 * EC, NCH], dt.bfloat16)
        ob = pp.tile([128, 1], dt.float32)

        nc.vector.memset(ones1[:, :], 1.0)
        nc.sync.dma_start(h0[:, :, :], h0T_d.rearrange("(k p) b -> p k b", p=128))
        nc.sync.dma_start(cT[:, :, :], c0T_d.rearrange("(k p) b -> p k b", p=128))
        nc.sync.dma_start(gb[:, :], gbias_d.rearrange("j p -> p j"))
        for k in range(EC):
            nc.sync.dma_start(whh[:, k, :, :], whh_d[k * 128:(k + 1) * 128])
            nc.sync.dma_start(
                ce[:, k, :, :],
                ceT_d[:, k * 128:(k + 1) * 128, :].rearrange("i p s -> p i s"),
            )
            nc.sync.dma_start(
                te[:, k, :, :],
                teT_d[:, k * 128:(k + 1) * 128, :].rearrange("i p s -> p i s"),
            )
        nc.sync.dma_start(oW[:, :, :], outWT_d.rearrange("(k p) n -> p k n", p=128))
        nc.sync.dma_start(ob[:, :], outb_d[:, None])

        # =================================================================
        # P1: XWT GEMM -> DRAM scratch; on-device weight folds; V'/te~ etc.
        # =================================================================
        with tc.tile_pool(name="p1", bufs=1) as p1, \
             tc.tile_pool(name="p1w", bufs=2) as p1w, \
             tc.tile_pool(name="ps1", bufs=3, space="PSUM") as ps1:
            wih = p1.tile([128, 16, 128], dt.bfloat16)
            nc.sync.dma_start(wih[:, :, :], wih_d[:, :, :])
            whhb = p1.tile([128, EC, 16, 128], dt.bfloat16)
            for k in range(EC):
                nc.sync.dma_start(whhb[:, k, :, :],
                                  whhb_d[k * 128:(k + 1) * 128])
            # lhsT tile sets [128, k, m, 128] and stream sets [128, k, 512]
            wqcL = p1.tile([128, EC, EC, 128], dt.bfloat16)
            wvcL = p1.tile([128, EC, EC, 128], dt.bfloat16)
            wktL = p1.tile([128, EC, EC, 128], dt.bfloat16)
            wvtL = p1.tile([128, EC, EC, 128], dt.bfloat16)
            wkcS = p1.tile([128, EC, E], dt.bfloat16)
            wocS = p1.tile([128, EC, E], dt.bfloat16)
            wqtS = p1.tile([128, EC, E], dt.bfloat16)
            wotS = p1.tile([128, EC, E], dt.bfloat16)
            for k in range(EC):
                for (dst, src) in ((wqcL, wqc_d), (wvcL, wvc_d),
                                   (wktL, wkt_d), (wvtL, wvt_d)):
                    nc.sync.dma_start(
                        dst[:, k, :, :],
                        src[k * 128:(k + 1) * 128, :]
                        .rearrange("p (m c) -> p m c", c=128),
                    )
            nc.sync.dma_start(wkcS[:, :, :], wkc_d.rearrange("(k p) e -> p k e", p=128))
            nc.sync.dma_start(wocS[:, :, :], wocT_d.rearrange("(k p) e -> p k e", p=128))
            nc.sync.dma_start(wqtS[:, :, :], wqt_d.rearrange("(k p) e -> p k e", p=128))
            nc.sync.dma_start(wotS[:, :, :], wotT_d.rearrange("(k p) e -> p k e", p=128))
            bqcC = p1.tile([128, EC, 1], dt.bfloat16)
            bvcC = p1.tile([128, EC, 1], dt.bfloat16)
            bqtC = p1.tile([128, EC, 1], dt.bfloat16)
            bvtC = p1.tile([128, EC, 1], dt.bfloat16)
            for (dst, src) in ((bqcC, bqc_d), (bvcC, bvc_d),
                               (bqtC, bqt_d), (bvtC, bvt_d)):
                nc.sync.dma_start(dst[:, :, :],
                                  src.rearrange("(k p) o -> p k o", p=128))
            bocS = p1.tile([128, EC], dt.float32)
            botS = p1.tile([128, EC], dt.float32)
            nc.sync.dma_start(bocS[:, :], boc_d.rearrange("(k p) -> p k", p=128))
            nc.sync.dma_start(botS[:, :], bot_d.rearrange("(k p) -> p k", p=128))

            mtT = p1.tile([128, EC, E], dt.bfloat16)       # M_t^T * QE [f, e]
            wvoc = p1.tile([128, EC, E], dt.bfloat16)      # (Wv^T Wo^T)_char
            wvot = p1.tile([128, EC, E], dt.bfloat16)
            vtC = p1.tile([128, EC, 1], dt.bfloat16)       # v_t col * QE

            # --- XWT = Wih^T @ x + gbias, spilled to DRAM (tile-major) ---
            ncc = max(1, Ts * PB // 512)
            tpc = Ts // ncc
            for cc in range(ncc):
                xbuf = p1w.tile([128, tpc, PB], dt.bfloat16, tag="xbuf")
                nc.sync.dma_start(xbuf[:, :, :],
                                  xT_d[:, cc * tpc:(cc + 1) * tpc, :])
                for j in range(16):
                    ps = ps1.tile([128, tpc, PB], dt.float32, tag="ps")
                    nc.tensor.matmul(ps[:, :, :], wih[:, j, :], xbuf[:, :, :])
                    stg = p1w.tile([128, tpc, PB], dt.bfloat16, tag="stg",
                                   bufs=4)
                    if j % 2 == 0:
                        nc.vector.tensor_scalar_add(stg[:, :, :], ps[:, :, :],
                                                    gb[:, j:j + 1])
                    else:
                        nc.scalar.activation(stg[:, :, :], ps[:, :, :],
                                             AF.Identity, bias=gb[:, j:j + 1])
                    nc.sync.dma_start(
                        xwt_d[j, :, cc * tpc:(cc + 1) * tpc, :], stg[:, :, :])

            # --- M_c = Wq_c^T @ Wk_c  (raw; QE applied at proj time) ---
            for m in range(EC):
                ps = ps1.tile([128, E], dt.float32, tag="ps")
                for k in range(EC):
                    nc.tensor.matmul(ps[:, :], wqcL[:, k, m, :], wkcS[:, k, :],
                                     start=(k == 0), stop=(k == EC - 1))
                for f2 in range(EC):
                    nc.any.tensor_copy(mc[:, m, f2, :],
                                       ps[:, f2 * 128:(f2 + 1) * 128])
            # --- v_c = (bq_c @ Wk_c) * QE  (per-partition bias for q~) ---
            for m in range(EC):
                ps = ps1.tile([128, 1], dt.float32, tag="ps")
                for k in range(EC):
                    nc.tensor.matmul(ps[:, :],
                                     wkcS[:, k, m * 128:(m + 1) * 128],
                                     bqcC[:, k, :],
                                     start=(k == 0), stop=(k == EC - 1))
                nc.scalar.activation(bqe[:, m:m + 1], ps[:, :], AF.Identity,
                                     scale=QE)
            # --- Wvo_c = Wv_c^T @ Wo_c^T ; badd_c = bv_c @ Wo_c^T + bo_c ---
            for m in range(EC):
                ps = ps1.tile([128, E], dt.float32, tag="ps")
                for k in range(EC):
                    nc.tensor.matmul(ps[:, :], wvcL[:, k, m, :], wocS[:, k, :],
                                     start=(k == 0), stop=(k == EC - 1))
                nc.any.tensor_copy(wvoc[:, m, :], ps[:, :])
            for m in range(EC):
                ps = ps1.tile([128, 1], dt.float32, tag="ps")
                for k in range(EC):
                    nc.tensor.matmul(ps[:, :],
                                     wocS[:, k, m * 128:(m + 1) * 128],
                                     bvcC[:, k, :],
                                     start=(k == 0), stop=(k == EC - 1))
                nc.scalar.activation(badc[:, m:m + 1], ps[:, :], AF.Identity,
                                     bias=bocS[:, m:m + 1])
            # --- tag: M_t^T*QE, v_t*QE, Wvo_t, badd_t ---
            for m in range(EC):
                ps = ps1.tile([128, E], dt.float32, tag="ps")
                for k in range(EC):
                    nc.tensor.matmul(ps[:, :], wktL[:, k, m, :], wqtS[:, k, :],
                                     start=(k == 0), stop=(k == EC - 1))
                nc.scalar.activation(mtT[:, m, :], ps[:, :], AF.Identity,
                                     scale=QE * 16.0)  # hT carries h/16
            for m in range(EC):
                ps = ps1.tile([128, 1], dt.float32, tag="ps")
                for k in range(EC):
                    nc.tensor.matmul(ps[:, :],
                                     wktL[:, k, m, :].rearrange("p c -> p c"),
                                     bqtC[:, k, :],
                                     start=(k == 0), stop=(k == EC - 1))
                nc.scalar.activation(vtC[:, m, :], ps[:, :], AF.Identity,
                                     scale=QE)
            for m in range(EC):
                ps = ps1.tile([128, E], dt.float32, tag="ps")
                for k in range(EC):
                    nc.tensor.matmul(ps[:, :], wvtL[:, k, m, :], wotS[:, k, :],
                                     start=(k == 0), stop=(k == EC - 1))
                nc.any.tensor_copy(wvot[:, m, :], ps[:, :])
            for m in range(EC):
                ps = ps1.tile([128, 1], dt.float32, tag="ps")
                for k in range(EC):
                    nc.tensor.matmul(ps[:, :],
                                     wotS[:, k, m * 128:(m + 1) * 128],
                                     bvtC[:, k, :],
                                     start=(k == 0), stop=(k == EC - 1))
                nc.scalar.activation(badt[:, m:m + 1], ps[:, :], AF.Identity,
                                     bias=botS[:, m:m + 1])

            # --- V'_char[s, i, g] ---
            for i in range(PB):
                for sc in range(SC // 128):
                    ps = ps1.tile([128, E], dt.float32, tag="ps")
                    for k in range(EC):
                        nc.tensor.matmul(
                            ps[:, :], ce[:, k, i, sc * 128:(sc + 1) * 128],
                            wvoc[:, k, :],
                            start=(k == 0), stop=(k == EC - 1))
                    nc.any.tensor_copy(vcp[:, sc, i, :], ps[:, :])
            # --- V'_tag (partitions 0..31), then replicate to all 4 strips ---
            for i in range(PB):
                ps = ps1.tile([STG, E], dt.float32, tag="ps")
                for k in range(EC):
                    nc.tensor.matmul(ps[:, :], te[:, k, i, :], wvot[:, k, :],
                                     start=(k == 0), stop=(k == EC - 1))
                nc.any.tensor_copy(vtp[:STG, i, :], ps[:, :])
            for di in range(1, 4):
                nc.sync.dma_start(vtp[di * STG:(di + 1) * STG, :, :],
                                  vtp[0:STG, :, :])
            # --- te~^T[e, i, s] = M_t te^T (QE folded in mtT) ---
            for m in range(EC):
                ps = ps1.tile([128, PB * STG], dt.float32, tag="ps")
                for k in range(EC):
                    nc.tensor.matmul(
                        ps[:, :], mtT[:, k, m * 128:(m + 1) * 128],
                        te[:, k, :, :].rearrange("p i s -> p (i s)"),
                        start=(k == 0), stop=(k == EC - 1))
                nc.any.tensor_copy(te2[:, m, :, :].rearrange("p i s -> p (i s)"),
                                   ps[:, :])
            # --- beta[i, s] = v_t . te  (softmax-relevant part of bq_t) ---
            bps = ps1.tile([1, PB * STG], dt.float32, tag="ps")
            for k in range(EC):
                nc.tensor.matmul(bps[:, :], vtC[:, k, :],
                                 te[:, k, :, :].rearrange("p i s -> p (i s)"),
                                 start=(k == 0), stop=(k == EC - 1))
            nc.any.tensor_copy(beta[:, :], bps[:, :])

            # --- step-0 hidden-state gates in bf16 (h0 ~ N(0,1) is too
            # large for fp8 weight noise; everything later is tanh-bounded)
            g0ps = ps1.tile([128, 2, 4, 2, PB], dt.float32, tag="ps")
            for h in range(2):
                for k in range(EC):
                    for g in range(4):
                        for e2 in range(2):
                            # start/stop once per PSUM bank: start=True zeroes
                            # the whole 2KB region, so interleaved groups in
                            # one tile must share a single group
                            nc.tensor.matmul(
                                g0ps[:, h, g, e2, :],
                                whhb[:, k, h * 8 + g * 2 + e2, :],
                                h0[:, k, :],
                                start=(h == 0 and k == 0 and g == 0 and e2 == 0),
                                stop=(h == 1 and k == EC - 1 and g == 3
                                      and e2 == 1))
            nc.any.tensor_copy(g0[:, :, :, :, :], g0ps[:, :, :, :, :])

        # =================================================================
        # P2: sequential LSTM.  whh tile j = half*8 + gt*2 + e2,
        # gate order (i, f, o, g); feature chunk ec = 2*half + e2.
        # =================================================================
        with tc.tile_pool(name="p2w", bufs=3) as p2, \
             tc.tile_pool(name="p2s", bufs=2) as p2s, \
             tc.tile_pool(name="gp", bufs=2, space="PSUM") as gpp:
            slabs = {}

            def load_slab(w):
                sl = p2s.tile([128, SLAB, 2, 4, 2, PB], dt.bfloat16, tag="slab")
                for j in range(16):
                    nc.sync.dma_start(
                        sl[:, :, j // 8, (j // 2) % 4, j % 2, :],
                        xwt_d[j, :, w * SLAB:(w + 1) * SLAB, :])
                slabs[w] = sl

            load_slab(0)
            for t in range(Ts):
                w = t // SLAB
                if t % SLAB == 0 and w + 1 < Ts // SLAB:
                    load_slab(w + 1)
                slab = slabs[w]
                for h in range(2):
                    # each half gets its own PSUM bank: a bank's accumulation
                    # group must be fully closed before any engine reads it
                    gph = None
                    if t > 0:
                        gph = gpp.tile([128, 4, 2, PB], dt.float32,
                                       tag=f"gp{h}")
                        for k in range(EC):
                            rhs = hT[:, k, t - 1, :]
                            for g in range(4):
                                for e2 in range(2):
                                    nc.tensor.matmul(
                                        gph[:, g, e2, :],
                                        whh[:, k, h * 8 + g * 2 + e2, :], rhs,
                                        start=(k == 0 and g == 0 and e2 == 0),
                                        stop=(k == EC - 1 and g == 3
                                              and e2 == 1))
                    # cell update for this half; overlaps the other half's
                    # (and the next step's) matmuls
                    gsrc = gph if t > 0 else g0[:, h]
                    ga = p2.tile([128, 4, 2, PB], dt.float32, tag=f"ga{h}")
                    nc.vector.tensor_add(ga[:, :, :, :], gsrc[:, :, :, :],
                                         slab[:, t % SLAB, h, :, :, :])
                    if t == 0 and p2dbg:
                        nc.sync.dma_start(p2dbg["g0"][:, h], g0[:, h])
                        nc.sync.dma_start(p2dbg["sl"][:, h], slab[:, 0, h])
                        nc.sync.dma_start(p2dbg["ga"][:, h], ga[:, :, :, :])
                    sio = p2.tile([128, 3, 2, PB], dt.float32, tag=f"sio{h}")
                    tg = p2.tile([128, 2, PB], dt.float32, tag=f"tg{h}")
                    nc.scalar.activation(sio[:, :, :, :], ga[:, 0:3, :, :],
                                         AF.Sigmoid)
                    nc.scalar.activation(tg[:, :, :], ga[:, 3, :, :], AF.Tanh)
                    v = p2.tile([128, 2, PB], dt.float32, tag=f"v{h}")
                    u = p2.tile([128, 2, PB], dt.float32, tag=f"u{h}")
                    nc.vector.tensor_mul(v[:, :, :], sio[:, 1, :, :],
                                         cT[:, 2 * h:2 * h + 2, :])
                    nc.vector.tensor_mul(u[:, :, :], sio[:, 0, :, :],
                                         tg[:, :, :])
                    nc.vector.tensor_add(cT[:, 2 * h:2 * h + 2, :],
                                         u[:, :, :], v[:, :, :])
                    tcc = p2.tile([128, 2, PB], dt.float32, tag=f"tcc{h}")
                    nc.scalar.activation(tcc[:, :, :],
                                         cT[:, 2 * h:2 * h + 2, :], AF.Tanh)
                    # hT stores h/16: compensates the 16*Whh fp8e3 scaling
                    # on the next step's matmul (and via QE16 in P3)
                    nc.vector.scalar_tensor_tensor(
                        hT[:, 2 * h:2 * h + 2, t, :], sio[:, 2, :, :],
                        1.0 / 16.0, tcc[:, :, :], op0=AL.mult, op1=AL.mult)
                    if t == 0 and h == 1 and p2dbg:
                        nc.sync.dma_start(p2dbg["c1"][:, :, :], cT[:, :, :])
                if t % SLAB == SLAB - 1:
                    del slabs[w]

        if hdbg_d is not None:
            with tc.tile_pool(name="hdbg", bufs=2) as hdp:
                for k in range(EC):
                    hcp = hdp.tile([128, Ts, PB], dt.float32, tag="hc")
                    nc.any.tensor_copy(hcp[:, :, :], hT[:, k, :, :])
                    nc.sync.dma_start(hdbg_d[:, k, :, :], hcp[:, :, :])

        # =================================================================
        # P3: attention + output projection, per time block
        # =================================================================
        with tc.tile_pool(name="p3", bufs=1) as p3, \
             tc.tile_pool(name="p3w", bufs=2) as p3w, \
             tc.tile_pool(name="ps3", bufs=2, space="PSUM") as ps3, \
             tc.tile_pool(name="ps3b", bufs=2, space="PSUM") as ps3b:
            for blk in range(NBLK):
                t0 = blk * TB
                # ---- q~ = (h @ M_c + v_c) * QE, laid out [e, i, t] ----
                qT = p3.tile([128, EC, PB, TB], dt.bfloat16, tag="qT")
                tw = 512 // PB
                for m in range(EC):
                    for cc in range(TB // tw):
                        ps = ps3.tile([128, tw, PB], dt.float32, tag="ps")
                        for k in range(EC):
                            nc.tensor.matmul(
                                ps[:, :, :], mc[:, k, m, :],
                                hT[:, k, t0 + cc * tw:t0 + (cc + 1) * tw, :],
                                start=(k == 0), stop=(k == EC - 1))
                        nc.vector.tensor_scalar(
                            qT[:, m, :, :].rearrange("p i t -> p t i")
                            [:, cc * tw:(cc + 1) * tw, :],
                            ps[:, :, :], QE * 16.0, bqe[:, m:m + 1],
                            op0=AL.mult, op1=AL.add)

                # ---- char attention ----
                orc = p3.tile([128, EC, PB, TB], dt.bfloat16, tag="orc")
                for i in range(PB):
                    pc = ps3.tile([128, SC], dt.float32, tag="ps")
                    for k in range(EC):
                        nc.tensor.matmul(pc[:, :], qT[:, k, i, :],
                                         ce[:, k, i, :],
                                         start=(k == 0), stop=(k == EC - 1))
                    pe = p3w.tile([128, SC], dt.bfloat16, tag="pe")
                    dsum = p3w.tile([128, 1], dt.float32, tag="dsum")
                    nc.scalar.activation(pe[:, :], pc[:, :], AF.Exp,
                                         accum_out=dsum[:, :])
                    drec = p3w.tile([128, 1], dt.float32, tag="drec")
                    nc.vector.reciprocal(drec[:, :], dsum[:, :])
                    pn = p3w.tile([128, SC], dt.bfloat16, tag="pn")
                    nc.vector.tensor_scalar_mul(pn[:, :], pe[:, :],
                                                drec[:, 0:1])
                    pTt = p3w.tile([128, 2, TB], dt.bfloat16, tag="pTt")
                    for sc in range(2):
                        nc.sync.dma_start_transpose(
                            pTt[:, sc, :], pn[:, sc * 128:(sc + 1) * 128])
                    cps = ps3b.tile([128, EC, TB], dt.float32, tag="cps")
                    for m in range(EC):
                        for sc in range(2):
                            nc.tensor.matmul(
                                cps[:, m, :],
                                vcp[:, sc, i, m * 128:(m + 1) * 128],
                                pTt[:, sc, :],
                                start=(m == 0 and sc == 0),
                                stop=(m == EC - 1 and sc == 1))
                    for m in range(EC):
                        nc.vector.tensor_scalar(
                            orc[:, m, i, :], cps[:, m, :],
                            badc[:, m:m + 1], 0.0, op0=AL.add, op1=AL.max)

                # ---- tag attention ----
                ptp = ps3.tile([128, PB, STG], dt.float32, tag="pst")
                for i in range(PB):
                    for k in range(EC):
                        nc.tensor.matmul(
                            ptp[:, i, :], hT[:, k, t0:t0 + TB, i],
                            te2[:, k, i, :],
                            start=(i == 0 and k == 0), stop=False,
                            skip_group_check=True)
                nc.tensor.matmul(
                    ptp[:, :, :].rearrange("p i s -> p (i s)"),
                    ones1[:, :], beta[:, :],
                    start=False, stop=True, skip_group_check=True)
                pte = p3w.tile([128, PB, STG], dt.bfloat16, tag="pte")
                nc.scalar.activation(pte[:, :, :], ptp[:, :, :], AF.Exp)
                tsum = p3w.tile([128, PB], dt.float32, tag="tsum")
                nc.vector.reduce_sum(tsum[:, :], pte[:, :, :], axis=AX.X)
                trec = p3w.tile([128, PB], dt.float32, tag="trec")
                nc.vector.reciprocal(trec[:, :], tsum[:, :])
                ptn = p3w.tile([128, PB, STG], dt.bfloat16, tag="ptn")
                for i in range(PB):
                    nc.vector.tensor_scalar_mul(ptn[:, i, :], pte[:, i, :],
                                                trec[:, i:i + 1])
                ptT = p3w.tile([128, 4, TB], dt.bfloat16, tag="ptT")
                for ib in range(4):
                    nc.sync.dma_start_transpose(
                        ptT[:, ib, :],
                        ptn[:, 4 * ib:4 * ib + 4, :]
                        .rearrange("p i s -> p (i s)"))
                ort = p3.tile([128, EC, PB, TB], dt.bfloat16, tag="ort")
                for i in range(PB):
                    di, ib = i % 4, i // 4
                    cp2 = ps3b.tile([128, EC, TB], dt.float32, tag="cps")
                    for m in range(EC):
                        nc.tensor.matmul(
                            cp2[:, m, :],
                            vtp[di * STG:(di + 1) * STG, i,
                                m * 128:(m + 1) * 128],
                            ptT[di * STG:(di + 1) * STG, ib, :],
                            start=(m == 0), stop=(m == EC - 1),
                            tile_position=(di * STG, 0))
                    for m in range(EC):
                        nc.vector.tensor_scalar(
                            ort[:, m, i, :], cp2[:, m, :],
                            badt[:, m:m + 1], 0.0, op0=AL.add, op1=AL.max)

                # ---- output projection, written feature-major ----
                for grp in range(PB // 4):
                    ps = ps3.tile([128, 4 * TB], dt.float32, tag="ps")
                    for kk in range(2 * EC):
                        src = orc if kk < EC else ort
                        nc.tensor.matmul(
                            ps[:, :], oW[:, kk, :],
                            src[:, kk % EC, grp * 4:(grp + 1) * 4, :]
                            .rearrange("p i t -> p (i t)"),
                            start=(kk == 0), stop=(kk == 2 * EC - 1))
                    of = p3w.tile([128, 4, TB], dt.float32, tag="of")
                    nc.vector.tensor_scalar_add(
                        of[:, :, :].rearrange("p i t -> p (i t)"),
                        ps[:, :], ob[:, 0:1])
                    nc.sync.dma_start(
                        out_d[grp * 4:(grp + 1) * 4, :, t0:t0 + TB]
                        .rearrange("i n t -> n i t"),
                        of[:, :, :])

    nc.compile()
    return nc


def _prep_core(inputs, core, Ts=T):
    bf = ml_dtypes.bfloat16
    f8 = ml_dtypes.float8_e3m4
    s = slice(core * PB, (core + 1) * PB)
    ce = inputs["char_encoding"][s]
    teg = inputs["tag_encoding"][s]
    tos = inputs["true_output_seq"][s][:, :Ts]
    xs = np.concatenate(
        [np.zeros((PB, 1, NCH), np.float32), tos[:, 1:, :]], axis=1)
    # Whh/Wih rows: torch gate order (i,f,g,o) -> (i,f,o,g); feature chunk
    # ec split as (half, e2); tile j = half*8 + gt*2 + e2.
    W = inputs["lstm_Whh"].reshape(4, 4, 128, E)[[0, 1, 3, 2]]
    whhP = W.reshape(4, 2, 2, 128, E).transpose(4, 1, 0, 2, 3).reshape(E, 16, 128)
    V = inputs["lstm_Wih"].reshape(4, 4, 128, NCH)[[0, 1, 3, 2]]
    wihP = V.reshape(4, 2, 2, 128, NCH).transpose(4, 1, 0, 2, 3).reshape(NCH, 16, 128)
    gbv = (inputs["lstm_bih"] + inputs["lstm_bhh"]).reshape(4, 4, 128)[[0, 1, 3, 2]]
    gbias = gbv.reshape(4, 2, 2, 128).transpose(1, 0, 2, 3).reshape(16, 128)
    m = {
        "ceT": np.ascontiguousarray(ce.transpose(0, 2, 1)).astype(bf),
        "teT": np.ascontiguousarray(teg.transpose(0, 2, 1)).astype(bf),
        "xT": np.ascontiguousarray(xs.transpose(2, 1, 0)).astype(bf),
        "whhP": np.ascontiguousarray(whhP * 16.0).astype(f8),
        "whhB": np.ascontiguousarray(whhP).astype(bf),
        "wihP": np.ascontiguousarray(wihP).astype(bf),
        "gbias": np.ascontiguousarray(gbias).astype(np.float32),
        "wqc": inputs["ca_Wq"].astype(bf),
        "wkc": inputs["ca_Wk"].astype(bf),
        "wvc": inputs["ca_Wv"].astype(bf),
        "wocT": np.ascontiguousarray(inputs["ca_Wo"].T).astype(bf),
        "wqt": inputs["ta_Wq"].astype(bf),
        "wkt": inputs["ta_Wk"].astype(bf),
        "wvt": inputs["ta_Wv"].astype(bf),
        "wotT": np.ascontiguousarray(inputs["ta_Wo"].T).astype(bf),
        "bqc_col": inputs["ca_bq"][:, None].astype(bf),
        "bvc_col": inputs["ca_bv"][:, None].astype(bf),
        "bqt_col": inputs["ta_bq"][:, None].astype(bf),
        "bvt_col": inputs["ta_bv"][:, None].astype(bf),
        "boc": inputs["ca_bo"].astype(np.float32),
        "bot": inputs["ta_bo"].astype(np.float32),
        "outWT": np.ascontiguousarray(inputs["out_W"].T).astype(bf),
        "outb": inputs["out_b"].astype(np.float32),
        "h0T": np.ascontiguousarray(
            np.concatenate([inputs["char_hn"][0][s],
                            inputs["char_hn"][1][s]], -1).T).astype(bf),
        "c0T": np.ascontiguousarray(
            np.concatenate([inputs["char_cn"][0][s],
                            inputs["char_cn"][1][s]], -1).T).astype(np.float32),
    }
    return m


def kernel(**inputs):
    from concourse.bass_utils import run_bass_kernel_spmd

    inputs = {k: np.asarray(v, dtype=np.float32) for k, v in inputs.items()}
    if "nc" not in _cache:
        _cache["nc"] = _build(T)
    nc = _cache["nc"]
    in_maps = [_prep_core(inputs, c) for c in range(NCORES)]
    res = run_bass_kernel_spmd(nc, in_maps, list(range(NCORES)))
    _cache["last_res"] = res
    outs = [np.asarray(res.results[c]["out"]).transpose(0, 2, 1)
            for c in range(NCORES)]
    return np.ascontiguousarray(np.concatenate(outs, axis=0)).astype(np.float32)


# revision 32
# speedup vs baseline: 1.4324x; 1.0116x over previous
"""Trainium2 Bass kernel for nn_Decoder (LSTM decoder + dual attention).

Sharding: data-parallel over batch B=128 across 8 NeuronCores (16 samples each).
Feature-major on-chip layouts (features on partitions, time x batch on the free
dim).  Key optimizations over the naive structure:

  - xwt (X @ Wih^T) DRAM scratch stored gate-tile-major so both the P1 write
    and the P2 per-step read use >=512B contiguous runs (the naive layout
    produced 32B scatter descriptors and ran at ~4.5 GB/s).
  - Whh stored fp8e4m3: LDWEIGHTS with FWL runs ~2x faster than bf16, and the
    LSTM recurrence is weight-load-bound (64 self-loading 128x128 tiles per
    step, only 16 streaming columns each).  h stays bf16 (mixed-dtype matmul).
  - P2 emission order: per step, gate tiles are computed half-by-half
    (features 0:256 then 256:512) with the cell update of each half emitted
    immediately after its matmuls, so the elementwise tail of step t overlaps
    the leading matmuls of step t+1.
  - Attention algebra: softmax drops per-query constants, so
    scores = (h @ (Wq^T Wk) / sqrt(E) + bq Wk / sqrt(E)) @ ce^T  -- the K
    projection disappears.  Wo folds into V (V' = (ce Wv^T + bv) Wo^T), so the
    output projection of each attention disappears.  For the tag attention the
    fold goes into the encoding side (te~ = te M^T), which is 8x smaller than
    projecting all queries.
  - Probability transposes via DMA xbar transpose (no PE/PSUM round trip).
  - Output written feature-major ([B, NCH, T]) with 512B runs; the host does
    the final cheap transpose to [B, T, NCH].
"""

import contextlib

import numpy as np
import ml_dtypes

B, T, E, G, NCH, SC, STG = 128, 256, 512, 2048, 128, 256, 32
NCORES = 8
PB = B // NCORES  # per-core batch = 16
EC = E // 128     # E chunks = 4
SLAB = 16         # P2 xwt prefetch window (steps)

_cache = {}


def _build(Ts):
    import concourse.mybir as mybir
    from concourse import bacc
    from concourse.tile import TileContext

    dt = mybir.dt
    AF = mybir.ActivationFunctionType
    AX = mybir.AxisListType
    AL = mybir.AluOpType
    TB = min(128, Ts)            # P3 time-block size
    NBLK = Ts // TB
    QE = float(1.0 / np.sqrt(E))

    nc = bacc.Bacc(None, dynamic_dma_scratch_size=4096)

    def din(name, shape, d=dt.bfloat16):
        return nc.dram_tensor(name, shape, d, kind="ExternalInput")

    ceT_d = din("ceT", [PB, E, SC])
    teT_d = din("teT", [PB, E, STG])
    xT_d = din("xT", [NCH, Ts, PB])
    whh_d = din("whhP", [E, 16, 128], dt.float8e3)   # 16*Whh, e3m4
    whhb_d = din("whhB", [E, 16, 128])               # Whh bf16 (step-0 gates)
    wih_d = din("wihP", [NCH, 16, 128])
    gbias_d = din("gbias", [16, 128], dt.float32)
    # raw (untransposed, torch [out,in]) projection weights for on-device folds
    wqc_d = din("wqc", [E, E])
    wkc_d = din("wkc", [E, E])
    wvc_d = din("wvc", [E, E])
    wocT_d = din("wocT", [E, E])
    wqt_d = din("wqt", [E, E])
    wkt_d = din("wkt", [E, E])
    wvt_d = din("wvt", [E, E])
    wotT_d = din("wotT", [E, E])
    bqc_d = din("bqc_col", [E, 1])
    bvc_d = din("bvc_col", [E, 1])
    bqt_d = din("bqt_col", [E, 1])
    bvt_d = din("bvt_col", [E, 1])
    boc_d = din("boc", [E], dt.float32)
    bot_d = din("bot", [E], dt.float32)
    outWT_d = din("outWT", [2 * E, NCH])
    outb_d = din("outb", [NCH], dt.float32)
    h0T_d = din("h0T", [E, PB])
    c0T_d = din("c0T", [E, PB], dt.float32)

    xwt_d = nc.dram_tensor("xwt", [16, 128, Ts, PB], dt.bfloat16)
    out_d = nc.dram_tensor("out", [PB, NCH, Ts], dt.float32, kind="ExternalOutput")
    hdbg_d = None
    if _cache.get("debug_h"):
        hdbg_d = nc.dram_tensor("hdbg", [128, EC, Ts, PB], dt.float32,
                                kind="ExternalOutput")
    p2dbg = {}
    if _cache.get("debug_p2"):
        p2dbg["g0"] = nc.dram_tensor("g0dbg", [128, 2, 4, 2, PB], dt.float32,
                                     kind="ExternalOutput")
        p2dbg["sl"] = nc.dram_tensor("sldbg", [128, 2, 4, 2, PB], dt.bfloat16,
                                     kind="ExternalOutput")
        p2dbg["ga"] = nc.dram_tensor("gadbg", [128, 2, 4, 2, PB], dt.float32,
                                     kind="ExternalOutput")
        p2dbg["c1"] = nc.dram_tensor("c1dbg", [128, EC, PB], dt.float32,
                                     kind="ExternalOutput")

    with TileContext(nc) as tc, contextlib.ExitStack() as ctx:
        pp = ctx.enter_context(tc.tile_pool(name="persist", bufs=1))

        # ---- persistent tiles -------------------------------------------
        # hT split in two so step t+1's k=0/1 matmuls depend only on the
        # first half's cell update (precise cross-step dependencies)
        hTa = pp.tile([128, 2, Ts, PB], dt.bfloat16)
        hTb = pp.tile([128, 2, Ts, PB], dt.bfloat16)

        def hch(k):  # h chunk k -> (tile, sub-index)
            return (hTa, k) if k < 2 else (hTb, k - 2)
        cT = pp.tile([128, EC, PB], dt.float32)
        h0 = pp.tile([128, EC, PB], dt.bfloat16)
        ce = pp.tile([128, EC, PB, SC], dt.bfloat16)
        te = pp.tile([128, EC, PB, STG], dt.bfloat16)
        whh = pp.tile([128, EC, 16, 128], dt.float8e3)
        g0 = pp.tile([128, 2, 4, 2, PB], dt.float32)   # Whh @ h0 (bf16, P1)
        gb = pp.tile([128, 16], dt.float32)
        mc = pp.tile([128, EC, EC, 128], dt.bfloat16)      # M_c = Wq^T Wk tiles
        bqe = pp.tile([128, EC], dt.float32)               # v_c * QE
        vcp = pp.tile([128, 2, PB, E], dt.bfloat16)        # V'_char [s, i, g]
        vtp = pp.tile([128, PB, E], dt.bfloat16)           # V'_tag, 4x replicated
        te2 = pp.tile([128, EC, PB, STG], dt.bfloat16)     # te~^T [e, i, s]
        beta = pp.tile([1, PB * STG], dt.bfloat16)
        ones1 = pp.tile([1, 128], dt.bfloat16)
        badc = pp.tile([128, EC], dt.float32)              # bvo_c + bo_c
        badt = pp.tile([128, EC], dt.float32)
        oW = pp.tile([128, 2 * EC, NCH], dt.bfloat16)
        ob = pp.tile([128, 1], dt.float32)

        nc.vector.memset(ones1[:, :], 1.0)
        nc.sync.dma_start(h0[:, :, :], h0T_d.rearrange("(k p) b -> p k b", p=128))
        nc.sync.dma_start(cT[:, :, :], c0T_d.rearrange("(k p) b -> p k b", p=128))
        nc.sync.dma_start(gb[:, :], gbias_d.rearrange("j p -> p j"))
        nc.sync.dma_start(ob[:, :], outb_d[:, None])
        # bulk loads on the second (scalar) DMA queue so the sync queue can
        # feed the P1 GEMMs without waiting behind multi-MB transfers
        for k in range(EC):
            nc.scalar.dma_start(
                ce[:, k, :, :],
                ceT_d[:, k * 128:(k + 1) * 128, :].rearrange("i p s -> p i s"),
            )
            nc.scalar.dma_start(
                te[:, k, :, :],
                teT_d[:, k * 128:(k + 1) * 128, :].rearrange("i p s -> p i s"),
            )
        for k in range(EC):
            nc.scalar.dma_start(whh[:, k, :, :], whh_d[k * 128:(k + 1) * 128])
        nc.scalar.dma_start(oW[:, :, :], outWT_d.rearrange("(k p) n -> p k n", p=128))

        # =================================================================
        # P1: XWT GEMM -> DRAM scratch; on-device weight folds; V'/te~ etc.
        # =================================================================
        with tc.tile_pool(name="p1", bufs=1) as p1, \
             tc.tile_pool(name="p1w", bufs=2) as p1w, \
             tc.tile_pool(name="ps1", bufs=3, space="PSUM") as ps1:
            wih = p1.tile([128, 16, 128], dt.bfloat16)
            nc.sync.dma_start(wih[:, :, :], wih_d[:, :, :])
            whhb = p1.tile([128, EC, 16, 128], dt.bfloat16)
            for k in range(EC):
                nc.sync.dma_start(whhb[:, k, :, :],
                                  whhb_d[k * 128:(k + 1) * 128])
            # lhsT tile sets [128, k, m, 128] and stream sets [128, k, 512]
            wqcL = p1.tile([128, EC, EC, 128], dt.bfloat16)
            wvcL = p1.tile([128, EC, EC, 128], dt.bfloat16)
            wktL = p1.tile([128, EC, EC, 128], dt.bfloat16)
            wvtL = p1.tile([128, EC, EC, 128], dt.bfloat16)
            wkcS = p1.tile([128, EC, E], dt.bfloat16)
            wocS = p1.tile([128, EC, E], dt.bfloat16)
            wqtS = p1.tile([128, EC, E], dt.bfloat16)
            wotS = p1.tile([128, EC, E], dt.bfloat16)
            for k in range(EC):
                for (dst, src) in ((wqcL, wqc_d), (wvcL, wvc_d),
                                   (wktL, wkt_d), (wvtL, wvt_d)):
                    nc.sync.dma_start(
                        dst[:, k, :, :],
                        src[k * 128:(k + 1) * 128, :]
                        .rearrange("p (m c) -> p m c", c=128),
                    )
            nc.sync.dma_start(wkcS[:, :, :], wkc_d.rearrange("(k p) e -> p k e", p=128))
            nc.sync.dma_start(wocS[:, :, :], wocT_d.rearrange("(k p) e -> p k e", p=128))
            nc.sync.dma_start(wqtS[:, :, :], wqt_d.rearrange("(k p) e -> p k e", p=128))
            nc.sync.dma_start(wotS[:, :, :], wotT_d.rearrange("(k p) e -> p k e", p=128))
            bqcC = p1.tile([128, EC, 1], dt.bfloat16)
            bvcC = p1.tile([128, EC, 1], dt.bfloat16)
            bqtC = p1.tile([128, EC, 1], dt.bfloat16)
            bvtC = p1.tile([128, EC, 1], dt.bfloat16)
            for (dst, src) in ((bqcC, bqc_d), (bvcC, bvc_d),
                               (bqtC, bqt_d), (bvtC, bvt_d)):
                nc.sync.dma_start(dst[:, :, :],
                                  src.rearrange("(k p) o -> p k o", p=128))
            bocS = p1.tile([128, EC], dt.float32)
            botS = p1.tile([128, EC], dt.float32)
            nc.sync.dma_start(bocS[:, :], boc_d.rearrange("(k p) -> p k", p=128))
            nc.sync.dma_start(botS[:, :], bot_d.rearrange("(k p) -> p k", p=128))

            mtT = p1.tile([128, EC, E], dt.bfloat16)       # M_t^T * QE [f, e]
            wvoc = p1.tile([128, EC, E], dt.bfloat16)      # (Wv^T Wo^T)_char
            wvot = p1.tile([128, EC, E], dt.bfloat16)
            vtC = p1.tile([128, EC, 1], dt.bfloat16)       # v_t col * QE

            # --- XWT = Wih^T @ x + gbias, spilled to DRAM (tile-major) ---
            ncc = max(1, Ts * PB // 512)
            tpc = Ts // ncc
            for cc in range(ncc):
                xbuf = p1w.tile([128, tpc, PB], dt.bfloat16, tag="xbuf")
                nc.sync.dma_start(xbuf[:, :, :],
                                  xT_d[:, cc * tpc:(cc + 1) * tpc, :])
                for j in range(16):
                    ps = ps1.tile([128, tpc, PB], dt.float32, tag="ps")
                    nc.tensor.matmul(ps[:, :, :], wih[:, j, :], xbuf[:, :, :])
                    stg = p1w.tile([128, tpc, PB], dt.bfloat16, tag="stg",
                                   bufs=4)
                    if j % 2 == 0:
                        nc.vector.tensor_scalar_add(stg[:, :, :], ps[:, :, :],
                                                    gb[:, j:j + 1])
                    else:
                        nc.scalar.activation(stg[:, :, :], ps[:, :, :],
                                             AF.Identity, bias=gb[:, j:j + 1])
                    nc.sync.dma_start(
                        xwt_d[j, :, cc * tpc:(cc + 1) * tpc, :], stg[:, :, :])

            # --- M_c = Wq_c^T @ Wk_c  (raw; QE applied at proj time) ---
            for m in range(EC):
                ps = ps1.tile([128, E], dt.float32, tag="ps")
                for k in range(EC):
                    nc.tensor.matmul(ps[:, :], wqcL[:, k, m, :], wkcS[:, k, :],
                                     start=(k == 0), stop=(k == EC - 1))
                for f2 in range(EC):
                    nc.any.tensor_copy(mc[:, m, f2, :],
                                       ps[:, f2 * 128:(f2 + 1) * 128])
            # --- v_c = (bq_c @ Wk_c) * QE  (per-partition bias for q~) ---
            for m in range(EC):
                ps = ps1.tile([128, 1], dt.float32, tag="ps")
                for k in range(EC):
                    nc.tensor.matmul(ps[:, :],
                                     wkcS[:, k, m * 128:(m + 1) * 128],
                                     bqcC[:, k, :],
                                     start=(k == 0), stop=(k == EC - 1))
                nc.scalar.activation(bqe[:, m:m + 1], ps[:, :], AF.Identity,
                                     scale=QE)
            # --- Wvo_c = Wv_c^T @ Wo_c^T ; badd_c = bv_c @ Wo_c^T + bo_c ---
            for m in range(EC):
                ps = ps1.tile([128, E], dt.float32, tag="ps")
                for k in range(EC):
                    nc.tensor.matmul(ps[:, :], wvcL[:, k, m, :], wocS[:, k, :],
                                     start=(k == 0), stop=(k == EC - 1))
                nc.any.tensor_copy(wvoc[:, m, :], ps[:, :])
            for m in range(EC):
                ps = ps1.tile([128, 1], dt.float32, tag="ps")
                for k in range(EC):
                    nc.tensor.matmul(ps[:, :],
                                     wocS[:, k, m * 128:(m + 1) * 128],
                                     bvcC[:, k, :],
                                     start=(k == 0), stop=(k == EC - 1))
                nc.scalar.activation(badc[:, m:m + 1], ps[:, :], AF.Identity,
                                     bias=bocS[:, m:m + 1])
            # --- tag: M_t^T*QE, v_t*QE, Wvo_t, badd_t ---
            for m in range(EC):
                ps = ps1.tile([128, E], dt.float32, tag="ps")
                for k in range(EC):
                    nc.tensor.matmul(ps[:, :], wktL[:, k, m, :], wqtS[:, k, :],
                                     start=(k == 0), stop=(k == EC - 1))
                nc.scalar.activation(mtT[:, m, :], ps[:, :], AF.Identity,
                                     scale=QE * 16.0)  # hT carries h/16
            for m in range(EC):
                ps = ps1.tile([128, 1], dt.float32, tag="ps")
                for k in range(EC):
                    nc.tensor.matmul(ps[:, :],
                                     wktL[:, k, m, :].rearrange("p c -> p c"),
                                     bqtC[:, k, :],
                                     start=(k == 0), stop=(k == EC - 1))
                nc.scalar.activation(vtC[:, m, :], ps[:, :], AF.Identity,
                                     scale=QE)
            for m in range(EC):
                ps = ps1.tile([128, E], dt.float32, tag="ps")
                for k in range(EC):
                    nc.tensor.matmul(ps[:, :], wvtL[:, k, m, :], wotS[:, k, :],
                                     start=(k == 0), stop=(k == EC - 1))
                nc.any.tensor_copy(wvot[:, m, :], ps[:, :])
            for m in range(EC):
                ps = ps1.tile([128, 1], dt.float32, tag="ps")
                for k in range(EC):
                    nc.tensor.matmul(ps[:, :],
                                     wotS[:, k, m * 128:(m + 1) * 128],
                                     bvtC[:, k, :],
                                     start=(k == 0), stop=(k == EC - 1))
                nc.scalar.activation(badt[:, m:m + 1], ps[:, :], AF.Identity,
                                     bias=botS[:, m:m + 1])

            # --- V'_char[s, i, g] ---
            for i in range(PB):
                for sc in range(SC // 128):
                    ps = ps1.tile([128, E], dt.float32, tag="ps")
                    for k in range(EC):
                        nc.tensor.matmul(
                            ps[:, :], ce[:, k, i, sc * 128:(sc + 1) * 128],
                            wvoc[:, k, :],
                            start=(k == 0), stop=(k == EC - 1))
                    nc.any.tensor_copy(vcp[:, sc, i, :], ps[:, :])
            # --- V'_tag (partitions 0..31), then replicate to all 4 strips ---
            for i in range(PB):
                ps = ps1.tile([STG, E], dt.float32, tag="ps")
                for k in range(EC):
                    nc.tensor.matmul(ps[:, :], te[:, k, i, :], wvot[:, k, :],
                                     start=(k == 0), stop=(k == EC - 1))
                nc.any.tensor_copy(vtp[:STG, i, :], ps[:, :])
            for di in range(1, 4):
                nc.sync.dma_start(vtp[di * STG:(di + 1) * STG, :, :],
                                  vtp[0:STG, :, :])
            # --- te~^T[e, i, s] = M_t te^T (QE folded in mtT) ---
            for m in range(EC):
                ps = ps1.tile([128, PB * STG], dt.float32, tag="ps")
                for k in range(EC):
                    nc.tensor.matmul(
                        ps[:, :], mtT[:, k, m * 128:(m + 1) * 128],
                        te[:, k, :, :].rearrange("p i s -> p (i s)"),
                        start=(k == 0), stop=(k == EC - 1))
                nc.any.tensor_copy(te2[:, m, :, :].rearrange("p i s -> p (i s)"),
                                   ps[:, :])
            # --- beta[i, s] = v_t . te  (softmax-relevant part of bq_t) ---
            bps = ps1.tile([1, PB * STG], dt.float32, tag="ps")
            for k in range(EC):
                nc.tensor.matmul(bps[:, :], vtC[:, k, :],
                                 te[:, k, :, :].rearrange("p i s -> p (i s)"),
                                 start=(k == 0), stop=(k == EC - 1))
            nc.any.tensor_copy(beta[:, :], bps[:, :])

            # --- step-0 hidden-state gates in bf16 (h0 ~ N(0,1) is too
            # large for fp8 weight noise; everything later is tanh-bounded)
            g0ps = ps1.tile([128, 2, 4, 2, PB], dt.float32, tag="ps")
            for h in range(2):
                for k in range(EC):
                    for g in range(4):
                        for e2 in range(2):
                            # start/stop once per PSUM bank: start=True zeroes
                            # the whole 2KB region, so interleaved groups in
                            # one tile must share a single group
                            nc.tensor.matmul(
                                g0ps[:, h, g, e2, :],
                                whhb[:, k, h * 8 + g * 2 + e2, :],
                                h0[:, k, :],
                                start=(h == 0 and k == 0 and g == 0 and e2 == 0),
                                stop=(h == 1 and k == EC - 1 and g == 3
                                      and e2 == 1))
            nc.any.tensor_copy(g0[:, :, :, :, :], g0ps[:, :, :, :, :])

        # =================================================================
        # P2: sequential LSTM.  whh tile j = half*8 + gt*2 + e2,
        # gate order (i, f, o, g); feature chunk ec = 2*half + e2.
        # =================================================================
        with tc.tile_pool(name="p2w", bufs=3) as p2, \
             tc.tile_pool(name="p2s", bufs=2) as p2s, \
             tc.tile_pool(name="gp", bufs=2, space="PSUM") as gpp:
            slabs = {}

            def load_slab(w):
                sl = p2s.tile([128, SLAB, 2, 4, 2, PB], dt.bfloat16, tag="slab")
                for j in range(16):
                    nc.sync.dma_start(
                        sl[:, :, j // 8, (j // 2) % 4, j % 2, :],
                        xwt_d[j, :, w * SLAB:(w + 1) * SLAB, :])
                slabs[w] = sl

            load_slab(0)
            for t in range(Ts):
                w = t // SLAB
                if t % SLAB == 0 and w + 1 < Ts // SLAB:
                    load_slab(w + 1)
                slab = slabs[w]
                for h in range(2):
                    # each half gets its own PSUM bank: a bank's accumulation
                    # group must be fully closed before any engine reads it
                    gph = None
                    if t > 0:
                        gph = gpp.tile([128, 4, 2, PB], dt.float32,
                                       tag=f"gp{h}")
                        for k in range(EC):
                            hk, ks = hch(k)
                            rhs = hk[:, ks, t - 1, :]
                            for g in range(4):
                                for e2 in range(2):
                                    nc.tensor.matmul(
                                        gph[:, g, e2, :],
                                        whh[:, k, h * 8 + g * 2 + e2, :], rhs,
                                        start=(k == 0 and g == 0 and e2 == 0),
                                        stop=(k == EC - 1 and g == 3
                                              and e2 == 1))
                    # cell update for this half; overlaps the other half's
                    # (and the next step's) matmuls
                    gsrc = gph if t > 0 else g0[:, h]
                    ga = p2.tile([128, 4, 2, PB], dt.float32, tag=f"ga{h}")
                    nc.vector.tensor_add(ga[:, :, :, :], gsrc[:, :, :, :],
                                         slab[:, t % SLAB, h, :, :, :])
                    if t == 0 and p2dbg:
                        nc.sync.dma_start(p2dbg["g0"][:, h], g0[:, h])
                        nc.sync.dma_start(p2dbg["sl"][:, h], slab[:, 0, h])
                        nc.sync.dma_start(p2dbg["ga"][:, h], ga[:, :, :, :])
                    sio = p2.tile([128, 3, 2, PB], dt.float32, tag=f"sio{h}")
                    tg = p2.tile([128, 2, PB], dt.float32, tag=f"tg{h}")
                    nc.scalar.activation(sio[:, :, :, :], ga[:, 0:3, :, :],
                                         AF.Sigmoid)
                    nc.scalar.activation(tg[:, :, :], ga[:, 3, :, :], AF.Tanh)
                    v = p2.tile([128, 2, PB], dt.float32, tag=f"v{h}")
                    u = p2.tile([128, 2, PB], dt.float32, tag=f"u{h}")
                    nc.vector.tensor_mul(v[:, :, :], sio[:, 1, :, :],
                                         cT[:, 2 * h:2 * h + 2, :])
                    nc.vector.tensor_mul(u[:, :, :], sio[:, 0, :, :],
                                         tg[:, :, :])
                    nc.vector.tensor_add(cT[:, 2 * h:2 * h + 2, :],
                                         u[:, :, :], v[:, :, :])
                    tcc = p2.tile([128, 2, PB], dt.float32, tag=f"tcc{h}")
                    nc.scalar.activation(tcc[:, :, :],
                                         cT[:, 2 * h:2 * h + 2, :], AF.Tanh)
                    # hT stores h/16: compensates the 16*Whh fp8e3 scaling
                    # on the next step's matmul (and via QE16 in P3)
                    nc.vector.scalar_tensor_tensor(
                        (hTa if h == 0 else hTb)[:, :, t, :], sio[:, 2, :, :],
                        1.0 / 16.0, tcc[:, :, :], op0=AL.mult, op1=AL.mult)
                    if t == 0 and h == 1 and p2dbg:
                        nc.sync.dma_start(p2dbg["c1"][:, :, :], cT[:, :, :])
                if t % SLAB == SLAB - 1:
                    del slabs[w]

        if hdbg_d is not None:
            with tc.tile_pool(name="hdbg", bufs=2) as hdp:
                for k in range(EC):
                    hk, ks = hch(k)
                    hcp = hdp.tile([128, Ts, PB], dt.float32, tag="hc")
                    nc.any.tensor_copy(hcp[:, :, :], hk[:, ks, :, :])
                    nc.sync.dma_start(hdbg_d[:, k, :, :], hcp[:, :, :])

        # =================================================================
        # P3: attention + output projection, per time block
        # =================================================================
        with tc.tile_pool(name="p3", bufs=1) as p3, \
             tc.tile_pool(name="p3w", bufs=2) as p3w, \
             tc.tile_pool(name="ps3", bufs=2, space="PSUM") as ps3, \
             tc.tile_pool(name="ps3b", bufs=2, space="PSUM") as ps3b:
            for blk in range(NBLK):
                t0 = blk * TB
                # ---- q~ = (h @ M_c + v_c) * QE, laid out [e, i, t] ----
                qT = p3.tile([128, EC, PB, TB], dt.bfloat16, tag="qT")
                tw = 512 // PB
                for m in range(EC):
                    for cc in range(TB // tw):
                        ps = ps3.tile([128, tw, PB], dt.float32, tag="ps")
                        for k in range(EC):
                            hk, ks = hch(k)
                            nc.tensor.matmul(
                                ps[:, :, :], mc[:, k, m, :],
                                hk[:, ks, t0 + cc * tw:t0 + (cc + 1) * tw, :],
                                start=(k == 0), stop=(k == EC - 1))
                        nc.vector.tensor_scalar(
                            qT[:, m, :, :].rearrange("p i t -> p t i")
                            [:, cc * tw:(cc + 1) * tw, :],
                            ps[:, :, :], QE * 16.0, bqe[:, m:m + 1],
                            op0=AL.mult, op1=AL.add)

                # ---- char attention ----
                orc = p3.tile([128, EC, PB, TB], dt.bfloat16, tag="orc")
                for i in range(PB):
                    pc = ps3.tile([128, SC], dt.float32, tag="ps")
                    for k in range(EC):
                        nc.tensor.matmul(pc[:, :], qT[:, k, i, :],
                                         ce[:, k, i, :],
                                         start=(k == 0), stop=(k == EC - 1))
                    pe = p3w.tile([128, SC], dt.bfloat16, tag="pe")
                    dsum = p3w.tile([128, 1], dt.float32, tag="dsum")
                    nc.scalar.activation(pe[:, :], pc[:, :], AF.Exp,
                                         accum_out=dsum[:, :])
                    drec = p3w.tile([128, 1], dt.float32, tag="drec")
                    nc.vector.reciprocal(drec[:, :], dsum[:, :])
                    pn = p3w.tile([128, SC], dt.bfloat16, tag="pn")
                    nc.vector.tensor_scalar_mul(pn[:, :], pe[:, :],
                                                drec[:, 0:1])
                    pTt = p3w.tile([128, 2, TB], dt.bfloat16, tag="pTt")
                    for sc in range(2):
                        nc.sync.dma_start_transpose(
                            pTt[:, sc, :], pn[:, sc * 128:(sc + 1) * 128])
                    cps = ps3b.tile([128, EC, TB], dt.float32, tag="cps")
                    for m in range(EC):
                        for sc in range(2):
                            nc.tensor.matmul(
                                cps[:, m, :],
                                vcp[:, sc, i, m * 128:(m + 1) * 128],
                                pTt[:, sc, :],
                                start=(m == 0 and sc == 0),
                                stop=(m == EC - 1 and sc == 1))
                    for m in range(EC):
                        nc.vector.tensor_scalar(
                            orc[:, m, i, :], cps[:, m, :],
                            badc[:, m:m + 1], 0.0, op0=AL.add, op1=AL.max)

                # ---- tag attention ----
                ptp = ps3.tile([128, PB, STG], dt.float32, tag="pst")
                for i in range(PB):
                    for k in range(EC):
                        hk, ks = hch(k)
                        nc.tensor.matmul(
                            ptp[:, i, :], hk[:, ks, t0:t0 + TB, i],
                            te2[:, k, i, :],
                            start=(i == 0 and k == 0), stop=False,
                            skip_group_check=True)
                nc.tensor.matmul(
                    ptp[:, :, :].rearrange("p i s -> p (i s)"),
                    ones1[:, :], beta[:, :],
                    start=False, stop=True, skip_group_check=True)
                pte = p3w.tile([128, PB, STG], dt.bfloat16, tag="pte")
                nc.scalar.activation(pte[:, :, :], ptp[:, :, :], AF.Exp)
                tsum = p3w.tile([128, PB], dt.float32, tag="tsum")
                nc.vector.reduce_sum(tsum[:, :], pte[:, :, :], axis=AX.X)
                trec = p3w.tile([128, PB], dt.float32, tag="trec")
                nc.vector.reciprocal(trec[:, :], tsum[:, :])
                ptn = p3w.tile([128, PB, STG], dt.bfloat16, tag="ptn")
                for i in range(PB):
                    nc.vector.tensor_scalar_mul(ptn[:, i, :], pte[:, i, :],
                                                trec[:, i:i + 1])
                ptT = p3w.tile([128, 4, TB], dt.bfloat16, tag="ptT")
                for ib in range(4):
                    nc.sync.dma_start_transpose(
                        ptT[:, ib, :],
                        ptn[:, 4 * ib:4 * ib + 4, :]
                        .rearrange("p i s -> p (i s)"))
                ort = p3.tile([128, EC, PB, TB], dt.bfloat16, tag="ort")
                for i in range(PB):
                    di, ib = i % 4, i // 4
                    cp2 = ps3b.tile([128, EC, TB], dt.float32, tag="cps")
                    for m in range(EC):
                        nc.tensor.matmul(
                            cp2[:, m, :],
                            vtp[di * STG:(di + 1) * STG, i,
                                m * 128:(m + 1) * 128],
                            ptT[di * STG:(di + 1) * STG, ib, :],
                            start=(m == 0), stop=(m == EC - 1),
                            tile_position=(di * STG, 0))
                    for m in range(EC):
                        nc.vector.tensor_scalar(
                            ort[:, m, i, :], cp2[:, m, :],
                            badt[:, m:m + 1], 0.0, op0=AL.add, op1=AL.max)

                # ---- output projection, written feature-major ----
                for grp in range(PB // 4):
                    ps = ps3.tile([128, 4 * TB], dt.float32, tag="ps")
                    for kk in range(2 * EC):
                        src = orc if kk < EC else ort
                        nc.tensor.matmul(
                            ps[:, :], oW[:, kk, :],
                            src[:, kk % EC, grp * 4:(grp + 1) * 4, :]
                            .rearrange("p i t -> p (i t)"),
                            start=(kk == 0), stop=(kk == 2 * EC - 1))
                    of = p3w.tile([128, 4, TB], dt.float32, tag="of")
                    nc.vector.tensor_scalar_add(
                        of[:, :, :].rearrange("p i t -> p (i t)"),
                        ps[:, :], ob[:, 0:1])
                    nc.sync.dma_start(
                        out_d[grp * 4:(grp + 1) * 4, :, t0:t0 + TB]
                        .rearrange("i n t -> n i t"),
                        of[:, :, :])

    nc.compile()
    return nc


def _prep_core(inputs, core, Ts=T):
    bf = ml_dtypes.bfloat16
    f8 = ml_dtypes.float8_e3m4
    s = slice(core * PB, (core + 1) * PB)
    ce = inputs["char_encoding"][s]
    teg = inputs["tag_encoding"][s]
    tos = inputs["true_output_seq"][s][:, :Ts]
    xs = np.concatenate(
        [np.zeros((PB, 1, NCH), np.float32), tos[:, 1:, :]], axis=1)
    # Whh/Wih rows: torch gate order (i,f,g,o) -> (i,f,o,g); feature chunk
    # ec split as (half, e2); tile j = half*8 + gt*2 + e2.
    W = inputs["lstm_Whh"].reshape(4, 4, 128, E)[[0, 1, 3, 2]]
    whhP = W.reshape(4, 2, 2, 128, E).transpose(4, 1, 0, 2, 3).reshape(E, 16, 128)
    V = inputs["lstm_Wih"].reshape(4, 4, 128, NCH)[[0, 1, 3, 2]]
    wihP = V.reshape(4, 2, 2, 128, NCH).transpose(4, 1, 0, 2, 3).reshape(NCH, 16, 128)
    gbv = (inputs["lstm_bih"] + inputs["lstm_bhh"]).reshape(4, 4, 128)[[0, 1, 3, 2]]
    gbias = gbv.reshape(4, 2, 2, 128).transpose(1, 0, 2, 3).reshape(16, 128)
    m = {
        "ceT": np.ascontiguousarray(ce.transpose(0, 2, 1)).astype(bf),
        "teT": np.ascontiguousarray(teg.transpose(0, 2, 1)).astype(bf),
        "xT": np.ascontiguousarray(xs.transpose(2, 1, 0)).astype(bf),
        "whhP": np.ascontiguousarray(whhP * 16.0).astype(f8),
        "whhB": np.ascontiguousarray(whhP).astype(bf),
        "wihP": np.ascontiguousarray(wihP).astype(bf),
        "gbias": np.ascontiguousarray(gbias).astype(np.float32),
        "wqc": inputs["ca_Wq"].astype(bf),
        "wkc": inputs["ca_Wk"].astype(bf),
        "wvc": inputs["ca_Wv"].astype(bf),
        "wocT": np.ascontiguousarray(inputs["ca_Wo"].T).astype(bf),
        "wqt": inputs["ta_Wq"].astype(bf),
        "wkt": inputs["ta_Wk"].astype(bf),
        "wvt": inputs["ta_Wv"].astype(bf),
        "wotT": np.ascontiguousarray(inputs["ta_Wo"].T).astype(bf),
        "bqc_col": inputs["ca_bq"][:, None].astype(bf),
        "bvc_col": inputs["ca_bv"][:, None].astype(bf),
        "bqt_col": inputs["ta_bq"][:, None].astype(bf),
        "bvt_col": inputs["ta_bv"][:, None].astype(bf),
        "boc": inputs["ca_bo"].astype(np.float32),
        "bot": inputs["ta_bo"].astype(np.float32),
        "outWT": np.ascontiguousarray(inputs["out_W"].T).astype(bf),
        "outb": inputs["out_b"].astype(np.float32),
        "h0T": np.ascontiguousarray(
            np.concatenate([inputs["char_hn"][0][s],
                            inputs["char_hn"][1][s]], -1).T).astype(bf),
        "c0T": np.ascontiguousarray(
            np.concatenate([inputs["char_cn"][0][s],
                            inputs["char_cn"][1][s]], -1).T).astype(np.float32),
    }
    return m


def kernel(**inputs):
    from concourse.bass_utils import run_bass_kernel_spmd

    inputs = {k: np.asarray(v, dtype=np.float32) for k, v in inputs.items()}
    if "nc" not in _cache:
        _cache["nc"] = _build(T)
    nc = _cache["nc"]
    in_maps = [_prep_core(inputs, c) for c in range(NCORES)]
    res = run_bass_kernel_spmd(nc, in_maps, list(range(NCORES)))
    _cache["last_res"] = res
    outs = [np.asarray(res.results[c]["out"]).transpose(0, 2, 1)
            for c in range(NCORES)]
    return np.ascontiguousarray(np.concatenate(outs, axis=0)).astype(np.float32)


# revision 35
# speedup vs baseline: 1.4639x; 1.0220x over previous
"""Trainium2 Bass kernel for nn_Decoder (LSTM decoder + dual attention).

Sharding: data-parallel over batch B=128 across 8 NeuronCores (16 samples each).
Feature-major on-chip layouts (features on partitions, time x batch free).

Structure:
  P1 (short): load weights, fold projection matrices on-device
     (M = Wq^T Wk / sqrt(E) so the K-projection disappears under softmax;
      Wvo = Wv^T Wo^T so the attention output projection folds into V),
     compute V'/te~/beta, step-0 gates in bf16, and the first xwt window.
  P2: 256 sequential LSTM steps.  Whh is fp8e3m4 (x16, h stored /16 in bf16):
     weight loads run 2x faster than bf16 and dominate the recurrence.
     The per-step cell update (DVE/ACT chain, ~2us) leaves the PE idle, so
     filler work is interleaved into each step's emission:
       - the x @ Wih^T GEMM for the slab window 16 steps ahead (SBUF-only,
         no DRAM scratch), and
       - from step 128 on, P3 attention units for the first time block.
  P3 tail: attention + output projection for the second time block.
"""

import contextlib

import numpy as np
import ml_dtypes

B, T, E, G, NCH, SC, STG = 128, 256, 512, 2048, 128, 256, 32
NCORES = 8
PB = B // NCORES  # per-core batch = 16
EC = E // 128     # E chunks = 4
SLAB = 16         # xwt window (steps)
GRP = 4           # P3 samples per group

_cache = {}


def _build(Ts):
    import concourse.mybir as mybir
    from concourse import bacc
    from concourse.tile import TileContext

    dt = mybir.dt
    AF = mybir.ActivationFunctionType
    AX = mybir.AxisListType
    AL = mybir.AluOpType
    TB = min(128, Ts)
    NBLK = Ts // TB
    NW = Ts // SLAB
    QE = float(1.0 / np.sqrt(E))
    QE16 = QE * 16.0

    nc = bacc.Bacc(None, dynamic_dma_scratch_size=4096)

    def din(name, shape, d=dt.bfloat16):
        return nc.dram_tensor(name, shape, d, kind="ExternalInput")

    ceT_d = din("ceT", [PB, E, SC])
    teT_d = din("teT", [PB, E, STG])
    xT_d = din("xT", [NCH, Ts, PB])
    whh_d = din("whhP", [E, 16, 128], dt.float8e3)   # 16*Whh, e3m4
    whhb_d = din("whhB", [E, 16, 128])               # Whh bf16 (step-0 gates)
    wih_d = din("wihP", [NCH, 16, 128])
    gbias_d = din("gbias", [16, 128], dt.float32)
    wqc_d = din("wqc", [E, E])
    wkc_d = din("wkc", [E, E])
    wvc_d = din("wvc", [E, E])
    wocT_d = din("wocT", [E, E])
    wqt_d = din("wqt", [E, E])
    wkt_d = din("wkt", [E, E])
    wvt_d = din("wvt", [E, E])
    wotT_d = din("wotT", [E, E])
    bqc_d = din("bqc_col", [E, 1])
    bvc_d = din("bvc_col", [E, 1])
    bqt_d = din("bqt_col", [E, 1])
    bvt_d = din("bvt_col", [E, 1])
    boc_d = din("boc", [E], dt.float32)
    bot_d = din("bot", [E], dt.float32)
    outWT_d = din("outWT", [2 * E, NCH])
    outb_d = din("outb", [NCH], dt.float32)
    h0T_d = din("h0T", [E, PB])
    c0T_d = din("c0T", [E, PB], dt.float32)

    out_d = nc.dram_tensor("out", [PB, NCH, Ts], dt.float32, kind="ExternalOutput")

    with TileContext(nc) as tc, contextlib.ExitStack() as ctx:
        pp = ctx.enter_context(tc.tile_pool(name="persist", bufs=1))
        p2w = ctx.enter_context(tc.tile_pool(name="p2w", bufs=3))
        p2s = ctx.enter_context(tc.tile_pool(name="p2s", bufs=2))
        gpp = ctx.enter_context(tc.tile_pool(name="gp", bufs=2, space="PSUM"))
        xwp = ctx.enter_context(tc.tile_pool(name="xwp", bufs=1, space="PSUM"))
        ps3 = ctx.enter_context(tc.tile_pool(name="ps3", bufs=2, space="PSUM"))
        cpp = ctx.enter_context(tc.tile_pool(name="cpp", bufs=1, space="PSUM"))

        # ---- persistent tiles -------------------------------------------
        hTa = pp.tile([128, 2, Ts, PB], dt.bfloat16)
        hTb = pp.tile([128, 2, Ts, PB], dt.bfloat16)

        def hch(k):
            return (hTa, k) if k < 2 else (hTb, k - 2)

        cT = pp.tile([128, EC, PB], dt.float32)
        h0 = pp.tile([128, EC, PB], dt.bfloat16)
        ce = pp.tile([128, EC, PB, SC], dt.bfloat16)
        te = pp.tile([128, EC, PB, STG], dt.bfloat16)
        whh = pp.tile([128, EC, 16, 128], dt.float8e3)
        g0 = pp.tile([128, 2, 4, 2, PB], dt.float32)
        wih = pp.tile([128, 16, 128], dt.bfloat16)
        gb = pp.tile([128, 16], dt.float32)
        mc = pp.tile([128, EC, EC, 128], dt.bfloat16)
        bqe = pp.tile([128, EC], dt.float32)
        vcp = pp.tile([128, 2, PB, E], dt.bfloat16)
        vtp = pp.tile([128, PB, E], dt.bfloat16)
        te2 = pp.tile([128, EC, PB, STG], dt.bfloat16)
        beta = pp.tile([1, PB, STG], dt.bfloat16)
        ones1 = pp.tile([1, 128], dt.bfloat16)
        badc = pp.tile([128, EC], dt.float32)
        badt = pp.tile([128, EC], dt.float32)
        oW = pp.tile([128, 2 * EC, NCH], dt.bfloat16)
        ob = pp.tile([128, 1], dt.float32)

        nc.vector.memset(ones1[:, :], 1.0)
        # sync queue: small init + wih + first x windows
        nc.sync.dma_start(h0[:, :, :], h0T_d.rearrange("(k p) b -> p k b", p=128))
        nc.sync.dma_start(cT[:, :, :], c0T_d.rearrange("(k p) b -> p k b", p=128))
        nc.sync.dma_start(gb[:, :], gbias_d.rearrange("j p -> p j"))
        nc.sync.dma_start(ob[:, :], outb_d[:, None])
        nc.sync.dma_start(wih[:, :, :], wih_d[:, :, :])

        xbufs, slabs = {}, {}

        def load_xbuf(w):
            xb = p2s.tile([128, SLAB, PB], dt.bfloat16, tag="xbuf", bufs=3)
            nc.sync.dma_start(xb[:, :, :], xT_d[:, w * SLAB:(w + 1) * SLAB, :])
            xbufs[w] = xb

        def xwt_unit(w, j):
            # one gate tile of the x @ Wih^T slab for window w (SBUF-only)
            if j == 0:
                slabs[w] = p2s.tile([128, SLAB, 2, 4, 2, PB], dt.bfloat16,
                                    tag="slab", name=f"slab{w}")
            ps = xwp.tile([128, SLAB, PB], dt.float32, tag="xw")
            nc.tensor.matmul(ps[:, :, :], wih[:, j, :], xbufs[w][:, :, :])
            dst = slabs[w][:, :, j // 8, (j // 2) % 4, j % 2, :]
            if j % 2 == 0:
                nc.vector.tensor_scalar_add(dst, ps[:, :, :], gb[:, j:j + 1])
            else:
                nc.scalar.activation(dst, ps[:, :, :], AF.Identity,
                                     bias=gb[:, j:j + 1])

        load_xbuf(0)
        load_xbuf(1)
        # scalar (second) DMA queue: P2 weights first, then fold inputs, bulk
        for k in range(EC):
            nc.scalar.dma_start(whh[:, k, :, :], whh_d[k * 128:(k + 1) * 128])
        for k in range(EC):
            nc.scalar.dma_start(
                te[:, k, :, :],
                teT_d[:, k * 128:(k + 1) * 128, :].rearrange("i p s -> p i s"))

        # =================================================================
        # P1: on-device weight folds + V'/te~/beta/g0 + xwt window 0
        # =================================================================
        with tc.tile_pool(name="p1", bufs=1) as p1:
            for j in range(16):
                xwt_unit(0, j)

            wvoc = p1.tile([128, EC, E], dt.bfloat16)
            wvot = p1.tile([128, EC, E], dt.bfloat16)
            mtT = p1.tile([128, EC, E], dt.bfloat16)
            vtC = p1.tile([128, EC, 1], dt.bfloat16)
            bqcC = p1.tile([128, EC, 1], dt.bfloat16)
            bvcC = p1.tile([128, EC, 1], dt.bfloat16)
            bqtC = p1.tile([128, EC, 1], dt.bfloat16)
            bvtC = p1.tile([128, EC, 1], dt.bfloat16)
            bocS = p1.tile([128, EC], dt.float32)
            botS = p1.tile([128, EC], dt.float32)
            for (dst, src) in ((bqcC, bqc_d), (bvcC, bvc_d),
                               (bqtC, bqt_d), (bvtC, bvt_d)):
                nc.scalar.dma_start(dst[:, :, :],
                                    src.rearrange("(k p) o -> p k o", p=128))
            nc.scalar.dma_start(bocS[:, :], boc_d.rearrange("(k p) -> p k", p=128))
            nc.scalar.dma_start(botS[:, :], bot_d.rearrange("(k p) -> p k", p=128))

            def fold_phase(wL_d, wS_d):
                wL = p1.tile([128, EC, EC, 128], dt.bfloat16, tag="wL", bufs=2)
                wS = p1.tile([128, EC, E], dt.bfloat16, tag="wS", bufs=2)
                for k in range(EC):
                    nc.scalar.dma_start(
                        wL[:, k, :, :],
                        wL_d[k * 128:(k + 1) * 128, :]
                        .rearrange("p (m c) -> p m c", c=128))
                nc.scalar.dma_start(wS[:, :, :],
                                    wS_d.rearrange("(k p) e -> p k e", p=128))
                return wL, wS

            # --- phase A: M_c = Wq_c^T Wk_c (raw) ; v_c*QE -> bqe ---
            wL, wS = fold_phase(wqc_d, wkc_d)
            for m in range(EC):
                ps = cpp.tile([128, E], dt.float32, tag="cps")
                for k in range(EC):
                    nc.tensor.matmul(ps[:, :], wL[:, k, m, :], wS[:, k, :],
                                     start=(k == 0), stop=(k == EC - 1))
                for f2 in range(EC):
                    nc.any.tensor_copy(mc[:, m, f2, :],
                                       ps[:, f2 * 128:(f2 + 1) * 128])
            for m in range(EC):
                ps = ps3.tile([128, 1], dt.float32, tag="pc")
                for k in range(EC):
                    nc.tensor.matmul(ps[:, :],
                                     wS[:, k, m * 128:(m + 1) * 128],
                                     bqcC[:, k, :],
                                     start=(k == 0), stop=(k == EC - 1))
                nc.scalar.activation(bqe[:, m:m + 1], ps[:, :], AF.Identity,
                                     scale=QE)
            # --- phase B: Wvo_c ; badd_c ---
            wL, wS = fold_phase(wvc_d, wocT_d)
            for m in range(EC):
                ps = cpp.tile([128, E], dt.float32, tag="cps")
                for k in range(EC):
                    nc.tensor.matmul(ps[:, :], wL[:, k, m, :], wS[:, k, :],
                                     start=(k == 0), stop=(k == EC - 1))
                nc.any.tensor_copy(wvoc[:, m, :], ps[:, :])
            for m in range(EC):
                ps = ps3.tile([128, 1], dt.float32, tag="pc")
                for k in range(EC):
                    nc.tensor.matmul(ps[:, :],
                                     wS[:, k, m * 128:(m + 1) * 128],
                                     bvcC[:, k, :],
                                     start=(k == 0), stop=(k == EC - 1))
                nc.scalar.activation(badc[:, m:m + 1], ps[:, :], AF.Identity,
                                     bias=bocS[:, m:m + 1])
            # --- phase C: M_t^T*QE16 ; v_t*QE ---
            wL, wS = fold_phase(wkt_d, wqt_d)
            for m in range(EC):
                ps = cpp.tile([128, E], dt.float32, tag="cps")
                for k in range(EC):
                    nc.tensor.matmul(ps[:, :], wL[:, k, m, :], wS[:, k, :],
                                     start=(k == 0), stop=(k == EC - 1))
                nc.scalar.activation(mtT[:, m, :], ps[:, :], AF.Identity,
                                     scale=QE16)  # hT carries h/16
            for m in range(EC):
                ps = ps3.tile([128, 1], dt.float32, tag="pc")
                for k in range(EC):
                    nc.tensor.matmul(ps[:, :], wL[:, k, m, :], bqtC[:, k, :],
                                     start=(k == 0), stop=(k == EC - 1))
                nc.scalar.activation(vtC[:, m, :], ps[:, :], AF.Identity,
                                     scale=QE)
            # --- phase D: Wvo_t ; badd_t ---
            wL, wS = fold_phase(wvt_d, wotT_d)
            for m in range(EC):
                ps = cpp.tile([128, E], dt.float32, tag="cps")
                for k in range(EC):
                    nc.tensor.matmul(ps[:, :], wL[:, k, m, :], wS[:, k, :],
                                     start=(k == 0), stop=(k == EC - 1))
                nc.any.tensor_copy(wvot[:, m, :], ps[:, :])
            for m in range(EC):
                ps = ps3.tile([128, 1], dt.float32, tag="pc")
                for k in range(EC):
                    nc.tensor.matmul(ps[:, :],
                                     wS[:, k, m * 128:(m + 1) * 128],
                                     bvtC[:, k, :],
                                     start=(k == 0), stop=(k == EC - 1))
                nc.scalar.activation(badt[:, m:m + 1], ps[:, :], AF.Identity,
                                     bias=botS[:, m:m + 1])

            # ce (bulk) + oW now; whhb (g0) last on the scalar queue
            for k in range(EC):
                nc.scalar.dma_start(
                    ce[:, k, :, :],
                    ceT_d[:, k * 128:(k + 1) * 128, :]
                    .rearrange("i p s -> p i s"))
            nc.scalar.dma_start(oW[:, :, :],
                                outWT_d.rearrange("(k p) n -> p k n", p=128))

            # --- te~^T[e, i, s] = M_t te^T ; beta[i, s] = v_t . te ---
            for m in range(EC):
                ps = cpp.tile([128, PB * STG], dt.float32, tag="cps")
                for k in range(EC):
                    nc.tensor.matmul(
                        ps[:, :], mtT[:, k, m * 128:(m + 1) * 128],
                        te[:, k, :, :].rearrange("p i s -> p (i s)"),
                        start=(k == 0), stop=(k == EC - 1))
                nc.any.tensor_copy(te2[:, m, :, :].rearrange("p i s -> p (i s)"),
                                   ps[:, :])
            bps = ps3.tile([1, PB * STG], dt.float32, tag="pc")
            for k in range(EC):
                nc.tensor.matmul(bps[:, :], vtC[:, k, :],
                                 te[:, k, :, :].rearrange("p i s -> p (i s)"),
                                 start=(k == 0), stop=(k == EC - 1))
            nc.any.tensor_copy(beta[:, :, :].rearrange("o i s -> o (i s)"),
                               bps[:, :])
            # --- V'_tag (replicated to 4 partition strips) ---
            for i in range(PB):
                ps = cpp.tile([STG, E], dt.float32, tag="cps")
                for k in range(EC):
                    nc.tensor.matmul(ps[:, :], te[:, k, i, :], wvot[:, k, :],
                                     start=(k == 0), stop=(k == EC - 1))
                nc.any.tensor_copy(vtp[:STG, i, :], ps[:, :])
            for di in range(1, 4):
                nc.sync.dma_start(vtp[di * STG:(di + 1) * STG, :, :],
                                  vtp[0:STG, :, :])
            # --- V'_char[s, i, g] ---
            for i in range(PB):
                for sc in range(SC // 128):
                    ps = cpp.tile([128, E], dt.float32, tag="cps")
                    for k in range(EC):
                        nc.tensor.matmul(
                            ps[:, :], ce[:, k, i, sc * 128:(sc + 1) * 128],
                            wvoc[:, k, :],
                            start=(k == 0), stop=(k == EC - 1))
                    nc.any.tensor_copy(vcp[:, sc, i, :], ps[:, :])
            # --- step-0 hidden gates in bf16 (h0 ~ N(0,1): too large for
            # fp8 weight noise; later h is tanh-bounded) ---
            g0ps = ps3.tile([128, 2, 4, 2, PB], dt.float32, tag="pc")
            for k in range(EC):
                whhb = p1.tile([128, 16, 128], dt.bfloat16, tag="whhb", bufs=2)
                nc.scalar.dma_start(whhb[:, :, :],
                                    whhb_d[k * 128:(k + 1) * 128])
                for h in range(2):
                    for g in range(4):
                        for e2 in range(2):
                            nc.tensor.matmul(
                                g0ps[:, h, g, e2, :],
                                whhb[:, h * 8 + g * 2 + e2, :],
                                h0[:, k, :],
                                start=(k == 0 and h == 0 and g == 0 and e2 == 0),
                                stop=(k == EC - 1 and h == 1 and g == 3
                                      and e2 == 1))
            nc.any.tensor_copy(g0[:, :, :, :, :], g0ps[:, :, :, :, :])

        # =================================================================
        # P3 unit builder (used interleaved for block 0, serial for block 1)
        # =================================================================
        p3 = ctx.enter_context(tc.tile_pool(name="p3", bufs=2))
        gtiles = {}

        def p3_group_units(blk, grp):
            t0 = blk * TB
            i0 = grp * GRP

            def u_qproj():
                qTg = p3.tile([128, EC, GRP, TB], dt.bfloat16, tag="qT")
                org = p3.tile([128, 2 * EC, GRP, TB], dt.bfloat16, tag="og")
                gtiles[(blk, grp)] = (qTg, org)
                for m in range(EC):
                    ps = cpp.tile([128, TB, GRP], dt.float32, tag="cps")
                    for k in range(EC):
                        hk, ks = hch(k)
                        nc.tensor.matmul(
                            ps[:, :, :], mc[:, k, m, :],
                            hk[:, ks, t0:t0 + TB, i0:i0 + GRP],
                            start=(k == 0), stop=(k == EC - 1))
                    nc.vector.tensor_scalar(
                        qTg[:, m, :, :].rearrange("p i t -> p t i"),
                        ps[:, :, :], QE16, bqe[:, m:m + 1],
                        op0=AL.mult, op1=AL.add)

            def u_tag():
                qTg, org = gtiles[(blk, grp)]
                ptp = ps3.tile([128, GRP, STG], dt.float32, tag="pc")
                for di in range(GRP):
                    for k in range(EC):
                        hk, ks = hch(k)
                        nc.tensor.matmul(
                            ptp[:, di, :], hk[:, ks, t0:t0 + TB, i0 + di],
                            te2[:, k, i0 + di, :],
                            start=(di == 0 and k == 0), stop=False,
                            skip_group_check=True)
                nc.tensor.matmul(
                    ptp[:, :, :].rearrange("p i s -> p (i s)"),
                    ones1[:, :],
                    beta[:, i0:i0 + GRP, :].rearrange("o i s -> o (i s)"),
                    start=False, stop=True, skip_group_check=True)
                pte = p3.tile([128, GRP, STG], dt.bfloat16, tag="pte")
                nc.scalar.activation(pte[:, :, :], ptp[:, :, :], AF.Exp)
                tsum = p3.tile([128, GRP], dt.float32, tag="tsum")
                nc.vector.reduce_sum(tsum[:, :], pte[:, :, :], axis=AX.X)
                trec = p3.tile([128, GRP], dt.float32, tag="trec")
                nc.vector.reciprocal(trec[:, :], tsum[:, :])
                ptn = p3.tile([128, GRP, STG], dt.bfloat16, tag="ptn")
                for di in range(GRP):
                    nc.vector.tensor_scalar_mul(ptn[:, di, :], pte[:, di, :],
                                                trec[:, di:di + 1])
                ptT = p3.tile([128, TB], dt.bfloat16, tag="ptT")
                nc.sync.dma_start_transpose(
                    ptT[:, :], ptn[:, :, :].rearrange("p i s -> p (i s)"))
                gtiles[(blk, grp, "ptT")] = ptT

            def u_score(di):
                def f():
                    qTg, org = gtiles[(blk, grp)]
                    i = i0 + di
                    pc = ps3.tile([128, SC], dt.float32, tag="pc")
                    for k in range(EC):
                        nc.tensor.matmul(pc[:, :], qTg[:, k, di, :],
                                         ce[:, k, i, :],
                                         start=(k == 0), stop=(k == EC - 1))
                    pe = p3.tile([128, SC], dt.bfloat16, tag="pe")
                    dsum = p3.tile([128, 1], dt.float32, tag="dsum")
                    nc.scalar.activation(pe[:, :], pc[:, :], AF.Exp,
                                         accum_out=dsum[:, :])
                    drec = p3.tile([128, 1], dt.float32, tag="drec")
                    nc.vector.reciprocal(drec[:, :], dsum[:, :])
                    pn = p3.tile([128, SC], dt.bfloat16, tag="pn")
                    nc.vector.tensor_scalar_mul(pn[:, :], pe[:, :],
                                                drec[:, 0:1])
                    pTt = p3.tile([128, 2, TB], dt.bfloat16, tag="pTt")
                    for sc in range(2):
                        nc.sync.dma_start_transpose(
                            pTt[:, sc, :], pn[:, sc * 128:(sc + 1) * 128])
                    gtiles[(blk, grp, di)] = pTt
                return f

            def u_ctx(di):
                def f():
                    qTg, org = gtiles[(blk, grp)]
                    pTt = gtiles[(blk, grp, di)]
                    ptT = gtiles[(blk, grp, "ptT")]
                    i = i0 + di
                    cps = cpp.tile([128, EC, TB], dt.float32, tag="cps")
                    for m in range(EC):
                        for sc in range(2):
                            nc.tensor.matmul(
                                cps[:, m, :],
                                vcp[:, sc, i, m * 128:(m + 1) * 128],
                                pTt[:, sc, :],
                                start=(m == 0 and sc == 0),
                                stop=(m == EC - 1 and sc == 1))
                    for m in range(EC):
                        nc.vector.tensor_scalar(
                            org[:, m, di, :], cps[:, m, :],
                            badc[:, m:m + 1], 0.0, op0=AL.add, op1=AL.max)
                    cp2 = cpp.tile([128, EC, TB], dt.float32, tag="cps")
                    for m in range(EC):
                        nc.tensor.matmul(
                            cp2[:, m, :],
                            vtp[di * STG:(di + 1) * STG, i,
                                m * 128:(m + 1) * 128],
                            ptT[di * STG:(di + 1) * STG, :],
                            start=(m == 0), stop=(m == EC - 1),
                            tile_position=(di * STG, 0))
                    for m in range(EC):
                        nc.vector.tensor_scalar(
                            org[:, EC + m, di, :], cp2[:, m, :],
                            badt[:, m:m + 1], 0.0, op0=AL.add, op1=AL.max)
                return f

            def u_out():
                qTg, org = gtiles[(blk, grp)]
                ps = cpp.tile([128, GRP, TB], dt.float32, tag="cps")
                for kk in range(2 * EC):
                    nc.tensor.matmul(
                        ps[:, :, :].rearrange("p i t -> p (i t)"), oW[:, kk, :],
                        org[:, kk, :, :].rearrange("p i t -> p (i t)"),
                        start=(kk == 0), stop=(kk == 2 * EC - 1))
                of = p3.tile([128, GRP, TB], dt.float32, tag="of")
                nc.vector.tensor_scalar_add(
                    of[:, :, :].rearrange("p i t -> p (i t)"),
                    ps[:, :, :].rearrange("p i t -> p (i t)"), ob[:, 0:1])
                nc.sync.dma_start(
                    out_d[i0:i0 + GRP, :, t0:t0 + TB]
                    .rearrange("i n t -> n i t"),
                    of[:, :, :])

            units = [u_qproj, u_tag]
            for di in range(GRP):
                units.append(u_score(di))
                units.append(u_ctx(di))
            units.append(u_out)
            return units

        # =================================================================
        # P2: sequential LSTM with interleaved filler work
        # =================================================================
        fillq = []
        for grp in range(PB // GRP):
            fillq.extend(p3_group_units(0, grp))

        for t in range(Ts):
            if t % SLAB == 0:
                v = t // SLAB
                if v + 2 < NW:
                    load_xbuf(v + 2)
            slab = slabs[t // SLAB]
            for h in range(2):
                gph = None
                if t > 0:
                    gph = gpp.tile([128, 4, 2, PB], dt.float32, tag=f"gp{h}")
                    for k in range(EC):
                        hk, ks = hch(k)
                        rhs = hk[:, ks, t - 1, :]
                        for g in range(4):
                            for e2 in range(2):
                                nc.tensor.matmul(
                                    gph[:, g, e2, :],
                                    whh[:, k, h * 8 + g * 2 + e2, :], rhs,
                                    start=(k == 0 and g == 0 and e2 == 0),
                                    stop=(k == EC - 1 and g == 3 and e2 == 1))
                gsrc = gph if t > 0 else g0[:, h]
                ga = p2w.tile([128, 4, 2, PB], dt.float32, tag=f"ga{h}")
                nc.vector.tensor_add(ga[:, :, :, :], gsrc[:, :, :, :],
                                     slab[:, t % SLAB, h, :, :, :])
                sio = p2w.tile([128, 3, 2, PB], dt.float32, tag=f"sio{h}")
                tg = p2w.tile([128, 2, PB], dt.float32, tag=f"tg{h}")
                nc.scalar.activation(sio[:, :, :, :], ga[:, 0:3, :, :],
                                     AF.Sigmoid)
                nc.scalar.activation(tg[:, :, :], ga[:, 3, :, :], AF.Tanh)
                v_ = p2w.tile([128, 2, PB], dt.float32, tag=f"v{h}")
                u_ = p2w.tile([128, 2, PB], dt.float32, tag=f"u{h}")
                nc.vector.tensor_mul(v_[:, :, :], sio[:, 1, :, :],
                                     cT[:, 2 * h:2 * h + 2, :])
                nc.vector.tensor_mul(u_[:, :, :], sio[:, 0, :, :], tg[:, :, :])
                nc.vector.tensor_add(cT[:, 2 * h:2 * h + 2, :],
                                     u_[:, :, :], v_[:, :, :])
                tcc = p2w.tile([128, 2, PB], dt.float32, tag=f"tcc{h}")
                nc.scalar.activation(tcc[:, :, :],
                                     cT[:, 2 * h:2 * h + 2, :], AF.Tanh)
                # hT stores h/16 (compensates the 16x fp8 Whh scaling)
                nc.vector.scalar_tensor_tensor(
                    (hTa if h == 0 else hTb)[:, :, t, :], sio[:, 2, :, :],
                    1.0 / 16.0, tcc[:, :, :], op0=AL.mult, op1=AL.mult)
            # filler: next slab window's XWT tile; P3 block-0 units once
            # their h block is complete
            if t < Ts - SLAB:
                xwt_unit(t // SLAB + 1, t % SLAB)
            if t >= TB + 2 and fillq:
                fillq.pop(0)()

        while fillq:
            fillq.pop(0)()
        for blk in range(1, NBLK):
            for grp in range(PB // GRP):
                for u in p3_group_units(blk, grp):
                    u()

    nc.compile()
    return nc


def _prep_core(inputs, core, Ts=T):
    bf = ml_dtypes.bfloat16
    f8 = ml_dtypes.float8_e3m4
    s = slice(core * PB, (core + 1) * PB)
    ce = inputs["char_encoding"][s]
    teg = inputs["tag_encoding"][s]
    tos = inputs["true_output_seq"][s][:, :Ts]
    xs = np.concatenate(
        [np.zeros((PB, 1, NCH), np.float32), tos[:, 1:, :]], axis=1)
    # Whh/Wih rows: torch gate order (i,f,g,o) -> (i,f,o,g); feature chunk
    # ec split as (half, e2); tile j = half*8 + gt*2 + e2.
    W = inputs["lstm_Whh"].reshape(4, 4, 128, E)[[0, 1, 3, 2]]
    whhP = W.reshape(4, 2, 2, 128, E).transpose(4, 1, 0, 2, 3).reshape(E, 16, 128)
    V = inputs["lstm_Wih"].reshape(4, 4, 128, NCH)[[0, 1, 3, 2]]
    wihP = V.reshape(4, 2, 2, 128, NCH).transpose(4, 1, 0, 2, 3).reshape(NCH, 16, 128)
    gbv = (inputs["lstm_bih"] + inputs["lstm_bhh"]).reshape(4, 4, 128)[[0, 1, 3, 2]]
    gbias = gbv.reshape(4, 2, 2, 128).transpose(1, 0, 2, 3).reshape(16, 128)
    m = {
        "ceT": np.ascontiguousarray(ce.transpose(0, 2, 1)).astype(bf),
        "teT": np.ascontiguousarray(teg.transpose(0, 2, 1)).astype(bf),
        "xT": np.ascontiguousarray(xs.transpose(2, 1, 0)).astype(bf),
        "whhP": np.ascontiguousarray(whhP * 16.0).astype(f8),
        "whhB": np.ascontiguousarray(whhP).astype(bf),
        "wihP": np.ascontiguousarray(wihP).astype(bf),
        "gbias": np.ascontiguousarray(gbias).astype(np.float32),
        "wqc": inputs["ca_Wq"].astype(bf),
        "wkc": inputs["ca_Wk"].astype(bf),
        "wvc": inputs["ca_Wv"].astype(bf),
        "wocT": np.ascontiguousarray(inputs["ca_Wo"].T).astype(bf),
        "wqt": inputs["ta_Wq"].astype(bf),
        "wkt": inputs["ta_Wk"].astype(bf),
        "wvt": inputs["ta_Wv"].astype(bf),
        "wotT": np.ascontiguousarray(inputs["ta_Wo"].T).astype(bf),
        "bqc_col": inputs["ca_bq"][:, None].astype(bf),
        "bvc_col": inputs["ca_bv"][:, None].astype(bf),
        "bqt_col": inputs["ta_bq"][:, None].astype(bf),
        "bvt_col": inputs["ta_bv"][:, None].astype(bf),
        "boc": inputs["ca_bo"].astype(np.float32),
        "bot": inputs["ta_bo"].astype(np.float32),
        "outWT": np.ascontiguousarray(inputs["out_W"].T).astype(bf),
        "outb": inputs["out_b"].astype(np.float32),
        "h0T": np.ascontiguousarray(
            np.concatenate([inputs["char_hn"][0][s],
                            inputs["char_hn"][1][s]], -1).T).astype(bf),
        "c0T": np.ascontiguousarray(
            np.concatenate([inputs["char_cn"][0][s],
                            inputs["char_cn"][1][s]], -1).T).astype(np.float32),
    }
    return m


def kernel(**inputs):
    from concourse.bass_utils import run_bass_kernel_spmd

    inputs = {k: np.asarray(v, dtype=np.float32) for k, v in inputs.items()}
    if "nc" not in _cache:
        _cache["nc"] = _build(T)
    nc = _cache["nc"]
    in_maps = [_prep_core(inputs, c) for c in range(NCORES)]
    res = run_bass_kernel_spmd(nc, in_maps, list(range(NCORES)))
    _cache["last_res"] = res
    outs = [np.asarray(res.results[c]["out"]).transpose(0, 2, 1)
            for c in range(NCORES)]
    return np.ascontiguousarray(np.concatenate(outs, axis=0)).astype(np.float32)


# revision 41
# speedup vs baseline: 1.5821x; 1.0807x over previous
"""Trainium2 Bass kernel for nn_Decoder (LSTM decoder + dual attention).

Sharding: data-parallel over batch B=128 across 8 NeuronCores (16 samples each).
Feature-major on-chip layouts (features on partitions, time x batch free).

Structure:
  P1 (short): load weights, fold projection matrices on-device
     (M = Wq^T Wk / sqrt(E) so the K-projection disappears under softmax;
      Wvo = Wv^T Wo^T so the attention output projection folds into V),
     compute V'/te~/beta, step-0 gates in bf16, and the first xwt window.
  P2: 256 sequential LSTM steps.  Whh is fp8e3m4 (x16, h stored /16 in bf16):
     weight loads run 2x faster than bf16 and dominate the recurrence.
     The per-step cell update (DVE/ACT chain, ~2us) leaves the PE idle, so
     filler work is interleaved into each step's emission:
       - the x @ Wih^T GEMM for the slab window 16 steps ahead (SBUF-only,
         no DRAM scratch), and
       - from step 128 on, P3 attention units for the first time block.
  P3 tail: attention + output projection for the second time block.
"""

import contextlib

import numpy as np
import ml_dtypes

B, T, E, G, NCH, SC, STG = 128, 256, 512, 2048, 128, 256, 32
NCORES = 8
PB = B // NCORES  # per-core batch = 16
EC = E // 128     # E chunks = 4
SLAB = 16         # xwt window (steps)
GRP = 4           # P3 samples per group

_cache = {}


def _build(Ts):
    import concourse.mybir as mybir
    from concourse import bacc
    from concourse.tile import TileContext

    dt = mybir.dt
    AF = mybir.ActivationFunctionType
    AX = mybir.AxisListType
    AL = mybir.AluOpType
    TB = min(128, Ts)
    NBLK = Ts // TB
    NW = Ts // SLAB
    QE = float(1.0 / np.sqrt(E))
    QE16 = QE * 16.0

    nc = bacc.Bacc(None, dynamic_dma_scratch_size=4096)

    def din(name, shape, d=dt.bfloat16):
        return nc.dram_tensor(name, shape, d, kind="ExternalInput")

    ceT_d = din("ceT", [PB, E, SC])
    teT_d = din("teT", [PB, E, STG])
    xT_d = din("xT", [NCH, Ts, PB])
    whh_d = din("whhP", [E, 16, 128], dt.float8e3)   # 16*Whh, e3m4
    whhb_d = din("whhB", [E, 16, 128])               # Whh bf16 (step-0 gates)
    wih_d = din("wihP", [NCH, 16, 128])
    gbias_d = din("gbias", [16, 128], dt.float32)
    wqc_d = din("wqc", [E, E])
    wkc_d = din("wkc", [E, E])
    wvc_d = din("wvc", [E, E])
    wocT_d = din("wocT", [E, E])
    wqt_d = din("wqt", [E, E])
    wkt_d = din("wkt", [E, E])
    wvt_d = din("wvt", [E, E])
    wotT_d = din("wotT", [E, E])
    bqc_d = din("bqc_col", [E, 1])
    bvc_d = din("bvc_col", [E, 1])
    bqt_d = din("bqt_col", [E, 1])
    bvt_d = din("bvt_col", [E, 1])
    boc_d = din("boc", [E], dt.float32)
    bot_d = din("bot", [E], dt.float32)
    outWT_d = din("outWT", [2 * E, NCH])
    outb_d = din("outb", [NCH], dt.float32)
    h0T_d = din("h0T", [E, PB])
    c0T_d = din("c0T", [E, PB], dt.float32)

    out_d = nc.dram_tensor("out", [PB, NCH, Ts], dt.float32, kind="ExternalOutput")

    with TileContext(nc) as tc, contextlib.ExitStack() as ctx:
        pp = ctx.enter_context(tc.tile_pool(name="persist", bufs=1))
        p2w = ctx.enter_context(tc.tile_pool(name="p2w", bufs=3))
        p2s = ctx.enter_context(tc.tile_pool(name="p2s", bufs=2))
        gpp = ctx.enter_context(tc.tile_pool(name="gp", bufs=2, space="PSUM"))
        ps3 = ctx.enter_context(tc.tile_pool(name="ps3", bufs=2, space="PSUM"))
        cpp = ctx.enter_context(tc.tile_pool(name="cpp", bufs=2, space="PSUM"))

        # ---- persistent tiles -------------------------------------------
        hTa = pp.tile([128, 2, Ts, PB], dt.bfloat16)
        hTb = pp.tile([128, 2, Ts, PB], dt.bfloat16)

        def hch(k):
            return (hTa, k) if k < 2 else (hTb, k - 2)

        cT = pp.tile([128, EC, PB], dt.float32)
        h0 = pp.tile([128, EC, PB], dt.bfloat16)
        ce = pp.tile([128, EC, PB, SC], dt.bfloat16)
        te = pp.tile([128, EC, PB, STG], dt.bfloat16)
        whh = pp.tile([128, EC, 16, 128], dt.float8e3)
        g0 = pp.tile([128, 2, 4, 2, PB], dt.float32)
        wih = pp.tile([128, 16, 128], dt.bfloat16)
        gb = pp.tile([128, 16], dt.float32)
        mc = pp.tile([128, EC, EC, 128], dt.bfloat16)
        bqe = pp.tile([128, EC], dt.float32)
        vcp = pp.tile([128, 2, PB, E], dt.bfloat16)
        vtp = pp.tile([128, PB, E], dt.bfloat16)
        te2 = pp.tile([128, EC, PB, STG], dt.bfloat16)
        beta = pp.tile([1, PB, STG], dt.bfloat16)
        ones1 = pp.tile([1, 128], dt.bfloat16)
        badc = pp.tile([128, EC], dt.float32)
        badt = pp.tile([128, EC], dt.float32)
        oW = pp.tile([128, 2 * EC, NCH], dt.bfloat16)
        ob = pp.tile([128, 1], dt.float32)

        nc.vector.memset(ones1[:, :], 1.0)
        # sync queue: small init + wih + first x windows
        nc.sync.dma_start(h0[:, :, :], h0T_d.rearrange("(k p) b -> p k b", p=128))
        nc.sync.dma_start(cT[:, :, :], c0T_d.rearrange("(k p) b -> p k b", p=128))
        nc.sync.dma_start(gb[:, :], gbias_d.rearrange("j p -> p j"))
        nc.sync.dma_start(ob[:, :], outb_d[:, None])
        nc.sync.dma_start(wih[:, :, :], wih_d[:, :, :])

        xbufs, slabs = {}, {}

        def load_xbuf(w):
            xb = p2s.tile([128, SLAB, PB], dt.bfloat16, tag="xbuf", bufs=3)
            nc.sync.dma_start(xb[:, :, :], xT_d[:, w * SLAB:(w + 1) * SLAB, :])
            xbufs[w] = xb

        def xwt_unit(w, j):
            # one gate tile of the x @ Wih^T slab for window w (SBUF-only)
            if j == 0:
                slabs[w] = p2s.tile([128, SLAB, 2, 4, 2, PB], dt.bfloat16,
                                    tag="slab", name=f"slab{w}")
            ps = ps3.tile([128, SLAB, PB], dt.float32, tag="pc")
            nc.tensor.matmul(ps[:, :, :], wih[:, j, :], xbufs[w][:, :, :])
            dst = slabs[w][:, :, j // 8, (j // 2) % 4, j % 2, :]
            if j % 2 == 0:
                nc.vector.tensor_scalar_add(dst, ps[:, :, :], gb[:, j:j + 1])
            else:
                nc.scalar.activation(dst, ps[:, :, :], AF.Identity,
                                     bias=gb[:, j:j + 1])

        load_xbuf(0)
        load_xbuf(1)
        # scalar (second) DMA queue: P2 weights first, then fold inputs, bulk
        for k in range(EC):
            nc.scalar.dma_start(whh[:, k, :, :], whh_d[k * 128:(k + 1) * 128])
        for k in range(EC):
            nc.scalar.dma_start(
                te[:, k, :, :],
                teT_d[:, k * 128:(k + 1) * 128, :].rearrange("i p s -> p i s"))

        # =================================================================
        # P1: on-device weight folds + V'/te~/beta/g0 + xwt window 0
        # =================================================================
        with tc.tile_pool(name="p1", bufs=1) as p1:
            for j in range(16):
                xwt_unit(0, j)

            wvoc = p1.tile([128, EC, E], dt.bfloat16)
            wvot = p1.tile([128, EC, E], dt.bfloat16)
            mtT = p1.tile([128, EC, E], dt.bfloat16)
            vtC = p1.tile([128, EC, 1], dt.bfloat16)
            bqcC = p1.tile([128, EC, 1], dt.bfloat16)
            bvcC = p1.tile([128, EC, 1], dt.bfloat16)
            bqtC = p1.tile([128, EC, 1], dt.bfloat16)
            bvtC = p1.tile([128, EC, 1], dt.bfloat16)
            bocS = p1.tile([128, EC], dt.float32)
            botS = p1.tile([128, EC], dt.float32)
            for (dst, src) in ((bqcC, bqc_d), (bvcC, bvc_d),
                               (bqtC, bqt_d), (bvtC, bvt_d)):
                nc.scalar.dma_start(dst[:, :, :],
                                    src.rearrange("(k p) o -> p k o", p=128))
            nc.scalar.dma_start(bocS[:, :], boc_d.rearrange("(k p) -> p k", p=128))
            nc.scalar.dma_start(botS[:, :], bot_d.rearrange("(k p) -> p k", p=128))

            def fold_phase(wL_d, wS_d):
                wL = p1.tile([128, EC, EC, 128], dt.bfloat16, tag="wL", bufs=2)
                wS = p1.tile([128, EC, E], dt.bfloat16, tag="wS", bufs=2)
                for k in range(EC):
                    nc.scalar.dma_start(
                        wL[:, k, :, :],
                        wL_d[k * 128:(k + 1) * 128, :]
                        .rearrange("p (m c) -> p m c", c=128))
                nc.scalar.dma_start(wS[:, :, :],
                                    wS_d.rearrange("(k p) e -> p k e", p=128))
                return wL, wS

            # --- phase A: M_c = Wq_c^T Wk_c (raw) ; v_c*QE -> bqe ---
            wL, wS = fold_phase(wqc_d, wkc_d)
            for m in range(EC):
                ps = cpp.tile([128, E], dt.float32, tag="cps")
                for k in range(EC):
                    nc.tensor.matmul(ps[:, :], wL[:, k, m, :], wS[:, k, :],
                                     start=(k == 0), stop=(k == EC - 1))
                for f2 in range(EC):
                    nc.any.tensor_copy(mc[:, m, f2, :],
                                       ps[:, f2 * 128:(f2 + 1) * 128])
            for m in range(EC):
                ps = ps3.tile([128, 1], dt.float32, tag="pc")
                for k in range(EC):
                    nc.tensor.matmul(ps[:, :],
                                     wS[:, k, m * 128:(m + 1) * 128],
                                     bqcC[:, k, :],
                                     start=(k == 0), stop=(k == EC - 1))
                nc.scalar.activation(bqe[:, m:m + 1], ps[:, :], AF.Identity,
                                     scale=QE)
            # --- phase B: Wvo_c ; badd_c ---
            wL, wS = fold_phase(wvc_d, wocT_d)
            for m in range(EC):
                ps = cpp.tile([128, E], dt.float32, tag="cps")
                for k in range(EC):
                    nc.tensor.matmul(ps[:, :], wL[:, k, m, :], wS[:, k, :],
                                     start=(k == 0), stop=(k == EC - 1))
                nc.any.tensor_copy(wvoc[:, m, :], ps[:, :])
            for m in range(EC):
                ps = ps3.tile([128, 1], dt.float32, tag="pc")
                for k in range(EC):
                    nc.tensor.matmul(ps[:, :],
                                     wS[:, k, m * 128:(m + 1) * 128],
                                     bvcC[:, k, :],
                                     start=(k == 0), stop=(k == EC - 1))
                nc.scalar.activation(badc[:, m:m + 1], ps[:, :], AF.Identity,
                                     bias=bocS[:, m:m + 1])
            # --- phase C: M_t^T*QE16 ; v_t*QE ---
            wL, wS = fold_phase(wkt_d, wqt_d)
            for m in range(EC):
                ps = cpp.tile([128, E], dt.float32, tag="cps")
                for k in range(EC):
                    nc.tensor.matmul(ps[:, :], wL[:, k, m, :], wS[:, k, :],
                                     start=(k == 0), stop=(k == EC - 1))
                nc.scalar.activation(mtT[:, m, :], ps[:, :], AF.Identity,
                                     scale=QE16)  # hT carries h/16
            for m in range(EC):
                ps = ps3.tile([128, 1], dt.float32, tag="pc")
                for k in range(EC):
                    nc.tensor.matmul(ps[:, :], wL[:, k, m, :], bqtC[:, k, :],
                                     start=(k == 0), stop=(k == EC - 1))
                nc.scalar.activation(vtC[:, m, :], ps[:, :], AF.Identity,
                                     scale=QE)
            # --- phase D: Wvo_t ; badd_t ---
            wL, wS = fold_phase(wvt_d, wotT_d)
            for m in range(EC):
                ps = cpp.tile([128, E], dt.float32, tag="cps")
                for k in range(EC):
                    nc.tensor.matmul(ps[:, :], wL[:, k, m, :], wS[:, k, :],
                                     start=(k == 0), stop=(k == EC - 1))
                nc.any.tensor_copy(wvot[:, m, :], ps[:, :])
            for m in range(EC):
                ps = ps3.tile([128, 1], dt.float32, tag="pc")
                for k in range(EC):
                    nc.tensor.matmul(ps[:, :],
                                     wS[:, k, m * 128:(m + 1) * 128],
                                     bvtC[:, k, :],
                                     start=(k == 0), stop=(k == EC - 1))
                nc.scalar.activation(badt[:, m:m + 1], ps[:, :], AF.Identity,
                                     bias=botS[:, m:m + 1])

            # --- step-0 hidden gates in bf16 (h0 ~ N(0,1): too large for
            # fp8 weight noise; later h is tanh-bounded) ---
            g0ps = ps3.tile([128, 2, 4, 2, PB], dt.float32, tag="pc")
            for k in range(EC):
                whhb = p1.tile([128, 16, 128], dt.bfloat16, tag="whhb", bufs=2)
                nc.scalar.dma_start(whhb[:, :, :],
                                    whhb_d[k * 128:(k + 1) * 128])
                for h in range(2):
                    for g in range(4):
                        for e2 in range(2):
                            nc.tensor.matmul(
                                g0ps[:, h, g, e2, :],
                                whhb[:, h * 8 + g * 2 + e2, :],
                                h0[:, k, :],
                                start=(k == 0 and h == 0 and g == 0 and e2 == 0),
                                stop=(k == EC - 1 and h == 1 and g == 3
                                      and e2 == 1))
            nc.any.tensor_copy(g0[:, :, :, :, :], g0ps[:, :, :, :, :])

            # ce (bulk) + oW now on the scalar queue
            for k in range(EC):
                nc.scalar.dma_start(
                    ce[:, k, :, :],
                    ceT_d[:, k * 128:(k + 1) * 128, :]
                    .rearrange("i p s -> p i s"))
            nc.scalar.dma_start(oW[:, :, :],
                                outWT_d.rearrange("(k p) n -> p k n", p=128))

            # --- te~^T[e, i, s] = M_t te^T ; beta[i, s] = v_t . te ---
            for m in range(EC):
                ps = cpp.tile([128, PB * STG], dt.float32, tag="cps")
                for k in range(EC):
                    nc.tensor.matmul(
                        ps[:, :], mtT[:, k, m * 128:(m + 1) * 128],
                        te[:, k, :, :].rearrange("p i s -> p (i s)"),
                        start=(k == 0), stop=(k == EC - 1))
                nc.any.tensor_copy(te2[:, m, :, :].rearrange("p i s -> p (i s)"),
                                   ps[:, :])
            bps = ps3.tile([1, PB * STG], dt.float32, tag="pc")
            for k in range(EC):
                nc.tensor.matmul(bps[:, :], vtC[:, k, :],
                                 te[:, k, :, :].rearrange("p i s -> p (i s)"),
                                 start=(k == 0), stop=(k == EC - 1))
            nc.any.tensor_copy(beta[:, :, :].rearrange("o i s -> o (i s)"),
                               bps[:, :])
            # --- V'_tag (replicated to 4 partition strips) ---
            for i in range(PB):
                ps = cpp.tile([STG, E], dt.float32, tag="cps")
                for k in range(EC):
                    nc.tensor.matmul(ps[:, :], te[:, k, i, :], wvot[:, k, :],
                                     start=(k == 0), stop=(k == EC - 1))
                nc.any.tensor_copy(vtp[:STG, i, :], ps[:, :])
            for di in range(1, 4):
                nc.sync.dma_start(vtp[di * STG:(di + 1) * STG, :, :],
                                  vtp[0:STG, :, :])
            # --- V'_char[s, i, g] ---
            for i in range(PB):
                for sc in range(SC // 128):
                    ps = cpp.tile([128, E], dt.float32, tag="cps")
                    for k in range(EC):
                        nc.tensor.matmul(
                            ps[:, :], ce[:, k, i, sc * 128:(sc + 1) * 128],
                            wvoc[:, k, :],
                            start=(k == 0), stop=(k == EC - 1))
                    nc.any.tensor_copy(vcp[:, sc, i, :], ps[:, :])

        # =================================================================
        # P3 unit builder (used interleaved for block 0, serial for block 1)
        # =================================================================
        p3 = ctx.enter_context(tc.tile_pool(name="p3", bufs=2))
        gtiles = {}

        def p3_group_units(blk, grp):
            t0 = blk * TB
            i0 = grp * GRP

            def u_qproj():
                qTg = p3.tile([128, EC, GRP, TB], dt.bfloat16, tag="qT")
                org = p3.tile([128, 2 * EC, GRP, TB], dt.bfloat16, tag="og")
                gtiles[(blk, grp)] = (qTg, org)
                for m in range(EC):
                    ps = cpp.tile([128, TB, GRP], dt.float32, tag="cps")
                    for k in range(EC):
                        hk, ks = hch(k)
                        nc.tensor.matmul(
                            ps[:, :, :], mc[:, k, m, :],
                            hk[:, ks, t0:t0 + TB, i0:i0 + GRP],
                            start=(k == 0), stop=(k == EC - 1))
                    nc.vector.tensor_scalar(
                        qTg[:, m, :, :].rearrange("p i t -> p t i"),
                        ps[:, :, :], QE16, bqe[:, m:m + 1],
                        op0=AL.mult, op1=AL.add)

            def u_tag():
                qTg, org = gtiles[(blk, grp)]
                ptp = ps3.tile([128, GRP, STG], dt.float32, tag="pc")
                for di in range(GRP):
                    for k in range(EC):
                        hk, ks = hch(k)
                        nc.tensor.matmul(
                            ptp[:, di, :], hk[:, ks, t0:t0 + TB, i0 + di],
                            te2[:, k, i0 + di, :],
                            start=(di == 0 and k == 0), stop=False,
                            skip_group_check=True)
                nc.tensor.matmul(
                    ptp[:, :, :].rearrange("p i s -> p (i s)"),
                    ones1[:, :],
                    beta[:, i0:i0 + GRP, :].rearrange("o i s -> o (i s)"),
                    start=False, stop=True, skip_group_check=True)
                pte = p3.tile([128, GRP, STG], dt.bfloat16, tag="pte")
                nc.scalar.activation(pte[:, :, :], ptp[:, :, :], AF.Exp)
                tsum = p3.tile([128, GRP], dt.float32, tag="tsum")
                nc.vector.reduce_sum(tsum[:, :], pte[:, :, :], axis=AX.X)
                trec = p3.tile([128, GRP], dt.float32, tag="trec")
                nc.vector.reciprocal(trec[:, :], tsum[:, :])
                ptn = p3.tile([128, GRP, STG], dt.bfloat16, tag="ptn")
                for di in range(GRP):
                    nc.vector.tensor_scalar_mul(ptn[:, di, :], pte[:, di, :],
                                                trec[:, di:di + 1])
                ptT = p3.tile([128, TB], dt.bfloat16, tag="ptT")
                nc.sync.dma_start_transpose(
                    ptT[:, :], ptn[:, :, :].rearrange("p i s -> p (i s)"))
                gtiles[(blk, grp, "ptT")] = ptT

            def u_score(di):
                def f():
                    qTg, org = gtiles[(blk, grp)]
                    i = i0 + di
                    pc = ps3.tile([128, SC], dt.float32, tag="pc")
                    for k in range(EC):
                        nc.tensor.matmul(pc[:, :], qTg[:, k, di, :],
                                         ce[:, k, i, :],
                                         start=(k == 0), stop=(k == EC - 1))
                    pe = p3.tile([128, SC], dt.bfloat16, tag="pe")
                    dsum = p3.tile([128, 1], dt.float32, tag="dsum")
                    nc.scalar.activation(pe[:, :], pc[:, :], AF.Exp,
                                         accum_out=dsum[:, :])
                    drec = p3.tile([128, 1], dt.float32, tag="drec")
                    nc.vector.reciprocal(drec[:, :], dsum[:, :])
                    pn = p3.tile([128, SC], dt.bfloat16, tag="pn")
                    nc.vector.tensor_scalar_mul(pn[:, :], pe[:, :],
                                                drec[:, 0:1])
                    pTt = p3.tile([128, 2, TB], dt.bfloat16, tag="pTt")
                    for sc in range(2):
                        nc.sync.dma_start_transpose(
                            pTt[:, sc, :], pn[:, sc * 128:(sc + 1) * 128])
                    gtiles[(blk, grp, di)] = pTt
                return f

            def u_ctx(di):
                def f():
                    qTg, org = gtiles[(blk, grp)]
                    pTt = gtiles[(blk, grp, di)]
                    ptT = gtiles[(blk, grp, "ptT")]
                    i = i0 + di
                    cps = cpp.tile([128, EC, TB], dt.float32, tag="cps")
                    for m in range(EC):
                        for sc in range(2):
                            nc.tensor.matmul(
                                cps[:, m, :],
                                vcp[:, sc, i, m * 128:(m + 1) * 128],
                                pTt[:, sc, :],
                                start=(m == 0 and sc == 0),
                                stop=(m == EC - 1 and sc == 1))
                    for m in range(EC):
                        nc.vector.tensor_scalar(
                            org[:, m, di, :], cps[:, m, :],
                            badc[:, m:m + 1], 0.0, op0=AL.add, op1=AL.max)
                    cp2 = cpp.tile([128, EC, TB], dt.float32, tag="cps")
                    for m in range(EC):
                        nc.tensor.matmul(
                            cp2[:, m, :],
                            vtp[di * STG:(di + 1) * STG, i,
                                m * 128:(m + 1) * 128],
                            ptT[di * STG:(di + 1) * STG, :],
                            start=(m == 0), stop=(m == EC - 1),
                            tile_position=(di * STG, 0))
                    for m in range(EC):
                        nc.vector.tensor_scalar(
                            org[:, EC + m, di, :], cp2[:, m, :],
                            badt[:, m:m + 1], 0.0, op0=AL.add, op1=AL.max)
                return f

            def u_out():
                qTg, org = gtiles[(blk, grp)]
                ps = cpp.tile([128, GRP, TB], dt.float32, tag="cps")
                for kk in range(2 * EC):
                    nc.tensor.matmul(
                        ps[:, :, :].rearrange("p i t -> p (i t)"), oW[:, kk, :],
                        org[:, kk, :, :].rearrange("p i t -> p (i t)"),
                        start=(kk == 0), stop=(kk == 2 * EC - 1))
                of = p3.tile([128, GRP, TB], dt.float32, tag="of")
                nc.vector.tensor_scalar_add(
                    of[:, :, :].rearrange("p i t -> p (i t)"),
                    ps[:, :, :].rearrange("p i t -> p (i t)"), ob[:, 0:1])
                nc.sync.dma_start(
                    out_d[i0:i0 + GRP, :, t0:t0 + TB]
                    .rearrange("i n t -> n i t"),
                    of[:, :, :])

            units = [u_qproj, u_tag]
            for di in range(GRP):
                units.append(u_score(di))
                units.append(u_ctx(di))
            units.append(u_out)
            return units

        # =================================================================
        # P2: sequential LSTM with interleaved filler work
        # =================================================================
        fillq = []
        for grp in range(PB // GRP):
            fillq.extend(p3_group_units(0, grp))

        for t in range(Ts):
            if t % SLAB == 0:
                v = t // SLAB
                if v + 2 < NW:
                    load_xbuf(v + 2)
            slab = slabs[t // SLAB]
            for h in range(2):
                gph = None
                if t > 0:
                    gph = gpp.tile([128, 4, 2, PB], dt.float32, tag=f"gp{h}")
                    for k in range(EC):
                        hk, ks = hch(k)
                        rhs = hk[:, ks, t - 1, :]
                        for g in range(4):
                            for e2 in range(2):
                                nc.tensor.matmul(
                                    gph[:, g, e2, :],
                                    whh[:, k, h * 8 + g * 2 + e2, :], rhs,
                                    start=(k == 0 and g == 0 and e2 == 0),
                                    stop=(k == EC - 1 and g == 3 and e2 == 1))
                gsrc = gph if t > 0 else g0[:, h]
                ga = p2w.tile([128, 4, 2, PB], dt.float32, tag=f"ga{h}")
                nc.vector.tensor_add(ga[:, :, :, :], gsrc[:, :, :, :],
                                     slab[:, t % SLAB, h, :, :, :])
                # g-gate rows are pre-scaled x2 host-side, so one sigmoid
                # covers all four gates: tanh(g) = 2*sigmoid(2g) - 1
                sio = p2w.tile([128, 4, 2, PB], dt.float32, tag=f"sio{h}")
                nc.scalar.activation(sio[:, :, :, :], ga[:, :, :, :],
                                     AF.Sigmoid)
                v_ = p2w.tile([128, 2, PB], dt.float32, tag=f"v{h}")
                a_ = p2w.tile([128, 2, PB], dt.float32, tag=f"a{h}")
                nc.vector.tensor_mul(v_[:, :, :], sio[:, 1, :, :],
                                     cT[:, 2 * h:2 * h + 2, :])
                # a = (sig(2g) - 0.5) * sig(i) = i*tanh(g)/2
                nc.vector.scalar_tensor_tensor(
                    a_[:, :, :], sio[:, 3, :, :], 0.5, sio[:, 0, :, :],
                    op0=AL.subtract, op1=AL.mult)
                nc.vector.scalar_tensor_tensor(
                    cT[:, 2 * h:2 * h + 2, :], a_[:, :, :], 2.0, v_[:, :, :],
                    op0=AL.mult, op1=AL.add)
                tcc = p2w.tile([128, 2, PB], dt.float32, tag=f"tcc{h}")
                nc.scalar.activation(tcc[:, :, :],
                                     cT[:, 2 * h:2 * h + 2, :], AF.Tanh)
                # hT stores h/16 (compensates the 16x fp8 Whh scaling)
                nc.vector.scalar_tensor_tensor(
                    (hTa if h == 0 else hTb)[:, :, t, :], sio[:, 2, :, :],
                    1.0 / 16.0, tcc[:, :, :], op0=AL.mult, op1=AL.mult)
            # filler: next slab window's XWT tile; P3 block-0 units once
            # their h block is complete
            if t < Ts - SLAB:
                xwt_unit(t // SLAB + 1, t % SLAB)
            if t >= TB + 2 and fillq:
                fillq.pop(0)()

        while fillq:
            fillq.pop(0)()
        for blk in range(1, NBLK):
            for grp in range(PB // GRP):
                for u in p3_group_units(blk, grp):
                    u()

    nc.compile()
    return nc


def _prep_core(inputs, core, Ts=T):
    bf = ml_dtypes.bfloat16
    f8 = ml_dtypes.float8_e3m4
    s = slice(core * PB, (core + 1) * PB)
    ce = inputs["char_encoding"][s]
    teg = inputs["tag_encoding"][s]
    tos = inputs["true_output_seq"][s][:, :Ts]
    xs = np.concatenate(
        [np.zeros((PB, 1, NCH), np.float32), tos[:, 1:, :]], axis=1)
    # Whh/Wih rows: torch gate order (i,f,g,o) -> (i,f,o,g); feature chunk
    # ec split as (half, e2); tile j = half*8 + gt*2 + e2.
    # g-gate rows x2: tanh(g) computed as 2*sigmoid(2g) - 1 on device
    W = inputs["lstm_Whh"].reshape(4, 4, 128, E)[[0, 1, 3, 2]].copy()
    W[3] *= 2.0
    whhP = W.reshape(4, 2, 2, 128, E).transpose(4, 1, 0, 2, 3).reshape(E, 16, 128)
    V = inputs["lstm_Wih"].reshape(4, 4, 128, NCH)[[0, 1, 3, 2]].copy()
    V[3] *= 2.0
    wihP = V.reshape(4, 2, 2, 128, NCH).transpose(4, 1, 0, 2, 3).reshape(NCH, 16, 128)
    gbv = (inputs["lstm_bih"] + inputs["lstm_bhh"]).reshape(4, 4, 128)[[0, 1, 3, 2]].copy()
    gbv[3] *= 2.0
    gbias = gbv.reshape(4, 2, 2, 128).transpose(1, 0, 2, 3).reshape(16, 128)
    m = {
        "ceT": np.ascontiguousarray(ce.transpose(0, 2, 1)).astype(bf),
        "teT": np.ascontiguousarray(teg.transpose(0, 2, 1)).astype(bf),
        "xT": np.ascontiguousarray(xs.transpose(2, 1, 0)).astype(bf),
        "whhP": np.ascontiguousarray(whhP * 16.0).astype(f8),
        "whhB": np.ascontiguousarray(whhP).astype(bf),
        "wihP": np.ascontiguousarray(wihP).astype(bf),
        "gbias": np.ascontiguousarray(gbias).astype(np.float32),
        "wqc": inputs["ca_Wq"].astype(bf),
        "wkc": inputs["ca_Wk"].astype(bf),
        "wvc": inputs["ca_Wv"].astype(bf),
        "wocT": np.ascontiguousarray(inputs["ca_Wo"].T).astype(bf),
        "wqt": inputs["ta_Wq"].astype(bf),
        "wkt": inputs["ta_Wk"].astype(bf),
        "wvt": inputs["ta_Wv"].astype(bf),
        "wotT": np.ascontiguousarray(inputs["ta_Wo"].T).astype(bf),
        "bqc_col": inputs["ca_bq"][:, None].astype(bf),
        "bvc_col": inputs["ca_bv"][:, None].astype(bf),
        "bqt_col": inputs["ta_bq"][:, None].astype(bf),
        "bvt_col": inputs["ta_bv"][:, None].astype(bf),
        "boc": inputs["ca_bo"].astype(np.float32),
        "bot": inputs["ta_bo"].astype(np.float32),
        "outWT": np.ascontiguousarray(inputs["out_W"].T).astype(bf),
        "outb": inputs["out_b"].astype(np.float32),
        "h0T": np.ascontiguousarray(
            np.concatenate([inputs["char_hn"][0][s],
                            inputs["char_hn"][1][s]], -1).T).astype(bf),
        "c0T": np.ascontiguousarray(
            np.concatenate([inputs["char_cn"][0][s],
                            inputs["char_cn"][1][s]], -1).T).astype(np.float32),
    }
    return m


def kernel(**inputs):
    from concourse.bass_utils import run_bass_kernel_spmd

    inputs = {k: np.asarray(v, dtype=np.float32) for k, v in inputs.items()}
    if "nc" not in _cache:
        _cache["nc"] = _build(T)
    nc = _cache["nc"]
    in_maps = [_prep_core(inputs, c) for c in range(NCORES)]
    res = run_bass_kernel_spmd(nc, in_maps, list(range(NCORES)))
    _cache["last_res"] = res
    outs = [np.asarray(res.results[c]["out"]).transpose(0, 2, 1)
            for c in range(NCORES)]
    return np.ascontiguousarray(np.concatenate(outs, axis=0)).astype(np.float32)


# revision 60
# speedup vs baseline: 1.6020x; 1.0126x over previous
"""Trainium2 Bass kernel for nn_Decoder (LSTM decoder + dual attention).

Sharding: data-parallel over batch B=128 across 8 NeuronCores (16 samples each).
Feature-major on-chip layouts (features on partitions, time x batch free).

Structure:
  P1 (short): load weights, fold projection matrices on-device
     (M = Wq^T Wk / sqrt(E) so the K-projection disappears under softmax;
      Wvo = Wv^T Wo^T so the attention output projection folds into V),
     compute V'/te~/beta, step-0 gates in bf16, and the first xwt window.
  P2: 256 sequential LSTM steps.  Whh is fp8e3m4 (x16, h stored /16 in bf16):
     weight loads run 2x faster than bf16 and dominate the recurrence.
     The per-step cell update (DVE/ACT chain, ~2us) leaves the PE idle, so
     filler work is interleaved into each step's emission:
       - the x @ Wih^T GEMM for the slab window 16 steps ahead (SBUF-only,
         no DRAM scratch), and
       - from step 128 on, P3 attention units for the first time block.
  P3 tail: attention + output projection for the second time block.
"""

import contextlib

import numpy as np
import ml_dtypes

B, T, E, G, NCH, SC, STG = 128, 256, 512, 2048, 128, 256, 32
NCORES = 8
PB = B // NCORES  # per-core batch = 16
EC = E // 128     # E chunks = 4
SLAB = 16         # xwt window (steps)
GRP = 4           # P3 samples per group

_cache = {}


def _build(Ts):
    import concourse.mybir as mybir
    from concourse import bacc
    from concourse.tile import TileContext

    dt = mybir.dt
    AF = mybir.ActivationFunctionType
    AX = mybir.AxisListType
    AL = mybir.AluOpType
    TB = min(128, Ts)
    NBLK = Ts // TB
    NW = Ts // SLAB
    QE = float(1.0 / np.sqrt(E))
    QE16 = QE * 16.0

    nc = bacc.Bacc(None, dynamic_dma_scratch_size=4096)

    def din(name, shape, d=dt.bfloat16):
        return nc.dram_tensor(name, shape, d, kind="ExternalInput")

    ceT_d = din("ceT", [PB, E, SC])
    teT_d = din("teT", [PB, E, STG])
    xT_d = din("xT", [NCH, Ts, PB])
    whh_d = din("whhP", [E, 16, 128], dt.float8e3)   # 16*Whh, e3m4
    whhb_d = din("whhB", [E, 16, 128])               # Whh bf16 (step-0 gates)
    wih_d = din("wihP", [NCH, 16, 128])
    gbias_d = din("gbias", [16, 128], dt.float32)
    wqc_d = din("wqc", [E, E])
    wkc_d = din("wkc", [E, E])
    wvc_d = din("wvc", [E, E])
    wocT_d = din("wocT", [E, E])
    wqt_d = din("wqt", [E, E])
    wkt_d = din("wkt", [E, E])
    wvt_d = din("wvt", [E, E])
    wotT_d = din("wotT", [E, E])
    bqc_d = din("bqc_col", [E, 1])
    bvc_d = din("bvc_col", [E, 1])
    bqt_d = din("bqt_col", [E, 1])
    bvt_d = din("bvt_col", [E, 1])
    boc_d = din("boc", [E], dt.float32)
    bot_d = din("bot", [E], dt.float32)
    outWT_d = din("outWT", [2 * E, NCH])
    outb_d = din("outb", [NCH], dt.float32)
    h0T_d = din("h0T", [E, PB])
    c0T_d = din("c0T", [E, PB], dt.float32)

    out_d = nc.dram_tensor("out", [PB, NCH, Ts], dt.float32, kind="ExternalOutput")

    with TileContext(nc) as tc, contextlib.ExitStack() as ctx:
        pp = ctx.enter_context(tc.tile_pool(name="persist", bufs=1))
        p2w = ctx.enter_context(tc.tile_pool(name="p2w", bufs=3))
        p2s = ctx.enter_context(tc.tile_pool(name="p2s", bufs=2))
        gpp = ctx.enter_context(tc.tile_pool(name="gp", bufs=2, space="PSUM"))
        ps3 = ctx.enter_context(tc.tile_pool(name="ps3", bufs=2, space="PSUM"))
        cpp = ctx.enter_context(tc.tile_pool(name="cpp", bufs=2, space="PSUM"))

        # ---- persistent tiles -------------------------------------------
        hTa = pp.tile([128, 2, Ts, PB], dt.bfloat16)
        hTb = pp.tile([128, 2, Ts, PB], dt.bfloat16)

        def hch(k):
            return (hTa, k) if k < 2 else (hTb, k - 2)

        cT = pp.tile([128, EC, PB], dt.float32)
        h0 = pp.tile([128, EC, PB], dt.bfloat16)
        ce = pp.tile([128, EC, PB, SC], dt.bfloat16)
        te = pp.tile([128, EC, PB, STG], dt.bfloat16)
        whh = pp.tile([128, EC, 16, 128], dt.float8e3)
        g0 = pp.tile([128, 2, 4, 2, PB], dt.float32)
        wih = pp.tile([128, 16, 128], dt.bfloat16)
        gb = pp.tile([128, 16], dt.float32)
        mc = pp.tile([128, EC, EC, 128], dt.bfloat16)
        bqe = pp.tile([128, EC], dt.float32)
        vcp = pp.tile([128, 2, PB, E], dt.bfloat16)
        vtp = pp.tile([128, PB, E], dt.bfloat16)
        te2 = pp.tile([128, EC, PB, STG], dt.bfloat16)
        beta = pp.tile([1, PB, STG], dt.bfloat16)
        ones1 = pp.tile([1, 128], dt.bfloat16)
        badc = pp.tile([128, EC], dt.float32)
        badt = pp.tile([128, EC], dt.float32)
        oW = pp.tile([128, 2 * EC, NCH], dt.bfloat16)
        ob = pp.tile([128, 1], dt.float32)

        nc.vector.memset(ones1[:, :], 1.0)
        # sync queue: small init + wih + first x windows
        nc.sync.dma_start(h0[:, :, :], h0T_d.rearrange("(k p) b -> p k b", p=128))
        nc.sync.dma_start(cT[:, :, :], c0T_d.rearrange("(k p) b -> p k b", p=128))
        nc.sync.dma_start(gb[:, :], gbias_d.rearrange("j p -> p j"))
        nc.sync.dma_start(ob[:, :], outb_d[:, None])
        nc.sync.dma_start(wih[:, :, :], wih_d[:, :, :])

        xbufs, slabs = {}, {}

        def load_xbuf(w):
            xb = p2s.tile([128, SLAB, PB], dt.bfloat16, tag="xbuf", bufs=3)
            nc.scalar.dma_start(xb[:, :, :],
                                xT_d[:, w * SLAB:(w + 1) * SLAB, :])
            xbufs[w] = xb

        def xwt_unit(w, j):
            # one gate tile of the x @ Wih^T slab for window w (SBUF-only)
            if j == 0:
                slabs[w] = p2s.tile([128, SLAB, 2, 4, 2, PB], dt.bfloat16,
                                    tag="slab", name=f"slab{w}")
            ps = ps3.tile([128, SLAB, PB], dt.float32, tag="pc")
            nc.tensor.matmul(ps[:, :, :], wih[:, j, :], xbufs[w][:, :, :])
            dst = slabs[w][:, :, j // 8, (j // 2) % 4, j % 2, :]
            if j % 2 == 0:
                nc.vector.tensor_scalar_add(dst, ps[:, :, :], gb[:, j:j + 1])
            else:
                nc.scalar.activation(dst, ps[:, :, :], AF.Identity,
                                     bias=gb[:, j:j + 1])

        load_xbuf(0)
        load_xbuf(1)
        # scalar (second) DMA queue: P2 weights first, then fold inputs, bulk
        for k in range(EC):
            nc.scalar.dma_start(whh[:, k, :, :], whh_d[k * 128:(k + 1) * 128])
        for k in range(EC):
            nc.scalar.dma_start(
                te[:, k, :, :],
                teT_d[:, k * 128:(k + 1) * 128, :].rearrange("i p s -> p i s"))

        # =================================================================
        # P1: on-device weight folds + V'/te~/beta/g0 + xwt window 0
        # =================================================================
        with tc.tile_pool(name="p1", bufs=1) as p1:
            for j in range(16):
                xwt_unit(0, j)

            wvoc = p1.tile([128, EC, E], dt.bfloat16)
            wvot = p1.tile([128, EC, E], dt.bfloat16)
            mtT = p1.tile([128, EC, E], dt.bfloat16)
            vtC = p1.tile([128, EC, 1], dt.bfloat16)
            bqcC = p1.tile([128, EC, 1], dt.bfloat16)
            bvcC = p1.tile([128, EC, 1], dt.bfloat16)
            bqtC = p1.tile([128, EC, 1], dt.bfloat16)
            bvtC = p1.tile([128, EC, 1], dt.bfloat16)
            bocS = p1.tile([128, EC], dt.float32)
            botS = p1.tile([128, EC], dt.float32)
            for (dst, src) in ((bqcC, bqc_d), (bvcC, bvc_d),
                               (bqtC, bqt_d), (bvtC, bvt_d)):
                nc.scalar.dma_start(dst[:, :, :],
                                    src.rearrange("(k p) o -> p k o", p=128))
            nc.scalar.dma_start(bocS[:, :], boc_d.rearrange("(k p) -> p k", p=128))
            nc.scalar.dma_start(botS[:, :], bot_d.rearrange("(k p) -> p k", p=128))

            def fold_phase(wL_d, wS_d):
                wL = p1.tile([128, EC, EC, 128], dt.bfloat16, tag="wL", bufs=2)
                wS = p1.tile([128, EC, E], dt.bfloat16, tag="wS", bufs=2)
                for k in range(EC):
                    nc.scalar.dma_start(
                        wL[:, k, :, :],
                        wL_d[k * 128:(k + 1) * 128, :]
                        .rearrange("p (m c) -> p m c", c=128))
                nc.scalar.dma_start(wS[:, :, :],
                                    wS_d.rearrange("(k p) e -> p k e", p=128))
                return wL, wS

            # --- phase A: M_c = Wq_c^T Wk_c (raw) ; v_c*QE -> bqe ---
            wL, wS = fold_phase(wqc_d, wkc_d)
            for m in range(EC):
                ps = cpp.tile([128, E], dt.float32, tag="cps")
                for k in range(EC):
                    nc.tensor.matmul(ps[:, :], wL[:, k, m, :], wS[:, k, :],
                                     start=(k == 0), stop=(k == EC - 1))
                for f2 in range(EC):
                    nc.any.tensor_copy(mc[:, m, f2, :],
                                       ps[:, f2 * 128:(f2 + 1) * 128])
            for m in range(EC):
                ps = ps3.tile([128, 1], dt.float32, tag="pc")
                for k in range(EC):
                    nc.tensor.matmul(ps[:, :],
                                     wS[:, k, m * 128:(m + 1) * 128],
                                     bqcC[:, k, :],
                                     start=(k == 0), stop=(k == EC - 1))
                nc.scalar.activation(bqe[:, m:m + 1], ps[:, :], AF.Identity,
                                     scale=QE)
            # --- phase B: Wvo_c ; badd_c ---
            wL, wS = fold_phase(wvc_d, wocT_d)
            for m in range(EC):
                ps = cpp.tile([128, E], dt.float32, tag="cps")
                for k in range(EC):
                    nc.tensor.matmul(ps[:, :], wL[:, k, m, :], wS[:, k, :],
                                     start=(k == 0), stop=(k == EC - 1))
                nc.any.tensor_copy(wvoc[:, m, :], ps[:, :])
            for m in range(EC):
                ps = ps3.tile([128, 1], dt.float32, tag="pc")
                for k in range(EC):
                    nc.tensor.matmul(ps[:, :],
                                     wS[:, k, m * 128:(m + 1) * 128],
                                     bvcC[:, k, :],
                                     start=(k == 0), stop=(k == EC - 1))
                nc.scalar.activation(badc[:, m:m + 1], ps[:, :], AF.Identity,
                                     bias=bocS[:, m:m + 1])
            # --- phase C: M_t^T*QE16 ; v_t*QE ---
            wL, wS = fold_phase(wkt_d, wqt_d)
            for m in range(EC):
                ps = cpp.tile([128, E], dt.float32, tag="cps")
                for k in range(EC):
                    nc.tensor.matmul(ps[:, :], wL[:, k, m, :], wS[:, k, :],
                                     start=(k == 0), stop=(k == EC - 1))
                nc.scalar.activation(mtT[:, m, :], ps[:, :], AF.Identity,
                                     scale=QE16)  # hT carries h/16
            for m in range(EC):
                ps = ps3.tile([128, 1], dt.float32, tag="pc")
                for k in range(EC):
                    nc.tensor.matmul(ps[:, :], wL[:, k, m, :], bqtC[:, k, :],
                                     start=(k == 0), stop=(k == EC - 1))
                nc.scalar.activation(vtC[:, m, :], ps[:, :], AF.Identity,
                                     scale=QE)
            # --- phase D: Wvo_t ; badd_t ---
            wL, wS = fold_phase(wvt_d, wotT_d)
            for m in range(EC):
                ps = cpp.tile([128, E], dt.float32, tag="cps")
                for k in range(EC):
                    nc.tensor.matmul(ps[:, :], wL[:, k, m, :], wS[:, k, :],
                                     start=(k == 0), stop=(k == EC - 1))
                nc.any.tensor_copy(wvot[:, m, :], ps[:, :])
            for m in range(EC):
                ps = ps3.tile([128, 1], dt.float32, tag="pc")
                for k in range(EC):
                    nc.tensor.matmul(ps[:, :],
                                     wS[:, k, m * 128:(m + 1) * 128],
                                     bvtC[:, k, :],
                                     start=(k == 0), stop=(k == EC - 1))
                nc.scalar.activation(badt[:, m:m + 1], ps[:, :], AF.Identity,
                                     bias=botS[:, m:m + 1])

            # --- step-0 hidden gates in bf16 (h0 ~ N(0,1): too large for
            # fp8 weight noise; later h is tanh-bounded) ---
            g0ps = ps3.tile([128, 2, 4, 2, PB], dt.float32, tag="pc")
            for k in range(EC):
                whhb = p1.tile([128, 16, 128], dt.bfloat16, tag="whhb", bufs=2)
                nc.scalar.dma_start(whhb[:, :, :],
                                    whhb_d[k * 128:(k + 1) * 128])
                for h in range(2):
                    for g in range(4):
                        for e2 in range(2):
                            nc.tensor.matmul(
                                g0ps[:, h, g, e2, :],
                                whhb[:, h * 8 + g * 2 + e2, :],
                                h0[:, k, :],
                                start=(k == 0 and h == 0 and g == 0 and e2 == 0),
                                stop=(k == EC - 1 and h == 1 and g == 3
                                      and e2 == 1))
            nc.any.tensor_copy(g0[:, :, :, :, :], g0ps[:, :, :, :, :])

            # ce (bulk) + oW now on the scalar queue
            for k in range(EC):
                nc.scalar.dma_start(
                    ce[:, k, :, :],
                    ceT_d[:, k * 128:(k + 1) * 128, :]
                    .rearrange("i p s -> p i s"))
            nc.scalar.dma_start(oW[:, :, :],
                                outWT_d.rearrange("(k p) n -> p k n", p=128))

            # --- te~^T[e, i, s] = M_t te^T ; beta[i, s] = v_t . te ---
            for m in range(EC):
                ps = cpp.tile([128, PB * STG], dt.float32, tag="cps")
                for k in range(EC):
                    nc.tensor.matmul(
                        ps[:, :], mtT[:, k, m * 128:(m + 1) * 128],
                        te[:, k, :, :].rearrange("p i s -> p (i s)"),
                        start=(k == 0), stop=(k == EC - 1))
                nc.any.tensor_copy(te2[:, m, :, :].rearrange("p i s -> p (i s)"),
                                   ps[:, :])
            bps = ps3.tile([1, PB * STG], dt.float32, tag="pc")
            for k in range(EC):
                nc.tensor.matmul(bps[:, :], vtC[:, k, :],
                                 te[:, k, :, :].rearrange("p i s -> p (i s)"),
                                 start=(k == 0), stop=(k == EC - 1))
            nc.any.tensor_copy(beta[:, :, :].rearrange("o i s -> o (i s)"),
                               bps[:, :])
            # --- V'_tag (replicated to 4 partition strips) ---
            for i in range(PB):
                ps = cpp.tile([STG, E], dt.float32, tag="cps")
                for k in range(EC):
                    nc.tensor.matmul(ps[:, :], te[:, k, i, :], wvot[:, k, :],
                                     start=(k == 0), stop=(k == EC - 1))
                nc.any.tensor_copy(vtp[:STG, i, :], ps[:, :])
            for di in range(1, 4):
                nc.sync.dma_start(vtp[di * STG:(di + 1) * STG, :, :],
                                  vtp[0:STG, :, :])
            # --- V'_char[s, i, g] ---
            for i in range(PB):
                for sc in range(SC // 128):
                    ps = cpp.tile([128, E], dt.float32, tag="cps")
                    for k in range(EC):
                        nc.tensor.matmul(
                            ps[:, :], ce[:, k, i, sc * 128:(sc + 1) * 128],
                            wvoc[:, k, :],
                            start=(k == 0), stop=(k == EC - 1))
                    nc.any.tensor_copy(vcp[:, sc, i, :], ps[:, :])

        # =================================================================
        # P3 unit builder (used interleaved for block 0, serial for block 1)
        # =================================================================
        p3 = ctx.enter_context(tc.tile_pool(name="p3", bufs=2))
        gtiles = {}

        def p3_group_units(blk, grp):
            t0 = blk * TB
            i0 = grp * GRP

            def u_qproj(m0):
                def f():
                    if m0 == 0:
                        qTg = p3.tile([128, EC, GRP, TB], dt.bfloat16,
                                      tag="qT", name="qTg")
                        org = p3.tile([128, 2 * EC, GRP, TB], dt.bfloat16,
                                      tag="og", name="org")
                        gtiles[(blk, grp)] = (qTg, org)
                    qTg, org = gtiles[(blk, grp)]
                    for m in (m0, m0 + 1):
                        ps = cpp.tile([128, TB, GRP], dt.float32, tag="cps")
                        for k in range(EC):
                            hk, ks = hch(k)
                            nc.tensor.matmul(
                                ps[:, :, :], mc[:, k, m, :],
                                hk[:, ks, t0:t0 + TB, i0:i0 + GRP],
                                start=(k == 0), stop=(k == EC - 1))
                        nc.vector.tensor_scalar(
                            qTg[:, m, :, :].rearrange("p i t -> p t i"),
                            ps[:, :, :], QE16, bqe[:, m:m + 1],
                            op0=AL.mult, op1=AL.add)
                return f

            def u_tag():
                qTg, org = gtiles[(blk, grp)]
                ptp = ps3.tile([128, GRP, STG], dt.float32, tag="pc")
                for di in range(GRP):
                    for k in range(EC):
                        hk, ks = hch(k)
                        nc.tensor.matmul(
                            ptp[:, di, :], hk[:, ks, t0:t0 + TB, i0 + di],
                            te2[:, k, i0 + di, :],
                            start=(di == 0 and k == 0), stop=False,
                            skip_group_check=True)
                nc.tensor.matmul(
                    ptp[:, :, :].rearrange("p i s -> p (i s)"),
                    ones1[:, :],
                    beta[:, i0:i0 + GRP, :].rearrange("o i s -> o (i s)"),
                    start=False, stop=True, skip_group_check=True)
                pte = p3.tile([128, GRP, STG], dt.bfloat16, tag="pte")
                nc.scalar.activation(pte[:, :, :], ptp[:, :, :], AF.Exp)
                tsum = p3.tile([128, GRP], dt.float32, tag="tsum")
                nc.vector.reduce_sum(tsum[:, :], pte[:, :, :], axis=AX.X)
                trec = p3.tile([128, GRP], dt.float32, tag="trec")
                nc.vector.reciprocal(trec[:, :], tsum[:, :])
                ptn = p3.tile([128, GRP, STG], dt.bfloat16, tag="ptn")
                for di in range(GRP):
                    nc.vector.tensor_scalar_mul(ptn[:, di, :], pte[:, di, :],
                                                trec[:, di:di + 1])
                ptT = p3.tile([128, TB], dt.bfloat16, tag="ptT")
                nc.sync.dma_start_transpose(
                    ptT[:, :], ptn[:, :, :].rearrange("p i s -> p (i s)"))
                gtiles[(blk, grp, "ptT")] = ptT

            def u_score(di):
                def f():
                    qTg, org = gtiles[(blk, grp)]
                    i = i0 + di
                    pc = ps3.tile([128, SC], dt.float32, tag="pc")
                    for k in range(EC):
                        nc.tensor.matmul(pc[:, :], qTg[:, k, di, :],
                                         ce[:, k, i, :],
                                         start=(k == 0), stop=(k == EC - 1))
                    pe = p3.tile([128, SC], dt.bfloat16, tag="pe")
                    dsum = p3.tile([128, 1], dt.float32, tag="dsum")
                    nc.scalar.activation(pe[:, :], pc[:, :], AF.Exp,
                                         accum_out=dsum[:, :])
                    drec = p3.tile([128, 1], dt.float32, tag="drec")
                    nc.vector.reciprocal(drec[:, :], dsum[:, :])
                    pn = p3.tile([128, SC], dt.bfloat16, tag="pn")
                    nc.vector.tensor_scalar_mul(pn[:, :], pe[:, :],
                                                drec[:, 0:1])
                    pTt = p3.tile([128, 2, TB], dt.bfloat16, tag="pTt")
                    for sc in range(2):
                        nc.sync.dma_start_transpose(
                            pTt[:, sc, :], pn[:, sc * 128:(sc + 1) * 128])
                    gtiles[(blk, grp, di)] = pTt
                return f

            def u_ctx(di):
                def f():
                    qTg, org = gtiles[(blk, grp)]
                    pTt = gtiles[(blk, grp, di)]
                    ptT = gtiles[(blk, grp, "ptT")]
                    i = i0 + di
                    cps = cpp.tile([128, EC, TB], dt.float32, tag="cps")
                    for m in range(EC):
                        for sc in range(2):
                            nc.tensor.matmul(
                                cps[:, m, :],
                                vcp[:, sc, i, m * 128:(m + 1) * 128],
                                pTt[:, sc, :],
                                start=(m == 0 and sc == 0),
                                stop=(m == EC - 1 and sc == 1))
                    for m in range(EC):
                        nc.vector.tensor_scalar(
                            org[:, m, di, :], cps[:, m, :],
                            badc[:, m:m + 1], 0.0, op0=AL.add, op1=AL.max)
                    cp2 = cpp.tile([128, EC, TB], dt.float32, tag="cps")
                    for m in range(EC):
                        nc.tensor.matmul(
                            cp2[:, m, :],
                            vtp[di * STG:(di + 1) * STG, i,
                                m * 128:(m + 1) * 128],
                            ptT[di * STG:(di + 1) * STG, :],
                            start=(m == 0), stop=(m == EC - 1),
                            tile_position=(di * STG, 0))
                    for m in range(EC):
                        nc.vector.tensor_scalar(
                            org[:, EC + m, di, :], cp2[:, m, :],
                            badt[:, m:m + 1], 0.0, op0=AL.add, op1=AL.max)
                return f

            def u_out():
                qTg, org = gtiles[(blk, grp)]
                ps = cpp.tile([128, GRP, TB], dt.float32, tag="cps")
                for kk in range(2 * EC):
                    nc.tensor.matmul(
                        ps[:, :, :].rearrange("p i t -> p (i t)"), oW[:, kk, :],
                        org[:, kk, :, :].rearrange("p i t -> p (i t)"),
                        start=(kk == 0), stop=(kk == 2 * EC - 1))
                of = p3.tile([128, GRP, TB], dt.float32, tag="of")
                nc.vector.tensor_scalar_add(
                    of[:, :, :].rearrange("p i t -> p (i t)"),
                    ps[:, :, :].rearrange("p i t -> p (i t)"), ob[:, 0:1])
                nc.sync.dma_start(
                    out_d[i0:i0 + GRP, :, t0:t0 + TB]
                    .rearrange("i n t -> n i t"),
                    of[:, :, :])

            units = [u_qproj(0), u_qproj(2), u_tag]
            for di in range(GRP):
                units.append(u_score(di))
                units.append(u_ctx(di))
            units.append(u_out)
            return units

        def p3_block_tail(blk):
            # pairwise group interleave: hides the DMA-transpose latency
            # between a sample's score and ctx units while keeping only two
            # groups' tiles live (pool bufs=2)
            for gp0 in range(0, PB // GRP, 2):
                ua = p3_group_units(blk, gp0)
                ub = p3_group_units(blk, gp0 + 1)
                for a, b in zip(ua, ub):
                    a()
                    b()

        # =================================================================
        # P2: sequential LSTM with interleaved filler work
        # =================================================================
        fillq = []
        for grp in range(PB // GRP):
            fillq.extend(p3_group_units(0, grp))

        for t in range(Ts):
            if t % SLAB == 0:
                v = t // SLAB
                if v + 2 < NW:
                    load_xbuf(v + 2)
            slab = slabs[t // SLAB]
            for h in range(2):
                gph = None
                if t > 0:
                    gph = gpp.tile([128, 4, 2, PB], dt.float32, tag=f"gp{h}")
                    for k in range(EC):
                        hk, ks = hch(k)
                        rhs = hk[:, ks, t - 1, :]
                        for g in range(4):
                            for e2 in range(2):
                                nc.tensor.matmul(
                                    gph[:, g, e2, :],
                                    whh[:, k, h * 8 + g * 2 + e2, :], rhs,
                                    start=(k == 0 and g == 0 and e2 == 0),
                                    stop=(k == EC - 1 and g == 3 and e2 == 1))
                gsrc = gph if t > 0 else g0[:, h]
                ga = p2w.tile([128, 4, 2, PB], dt.float32, tag=f"ga{h}")
                nc.vector.tensor_add(ga[:, :, :, :], gsrc[:, :, :, :],
                                     slab[:, t % SLAB, h, :, :, :])
                # g-gate rows are pre-scaled x2 host-side, so one sigmoid
                # covers all four gates: tanh(g) = 2*sigmoid(2g) - 1
                sio = p2w.tile([128, 4, 2, PB], dt.float32, tag=f"sio{h}")
                nc.scalar.activation(sio[:, :, :, :], ga[:, :, :, :],
                                     AF.Sigmoid)
                v_ = p2w.tile([128, 2, PB], dt.float32, tag=f"v{h}")
                a_ = p2w.tile([128, 2, PB], dt.float32, tag=f"a{h}")
                nc.vector.tensor_mul(v_[:, :, :], sio[:, 1, :, :],
                                     cT[:, 2 * h:2 * h + 2, :])
                # a = (sig(2g) - 0.5) * sig(i) = i*tanh(g)/2
                nc.vector.scalar_tensor_tensor(
                    a_[:, :, :], sio[:, 3, :, :], 0.5, sio[:, 0, :, :],
                    op0=AL.subtract, op1=AL.mult)
                nc.vector.scalar_tensor_tensor(
                    cT[:, 2 * h:2 * h + 2, :], a_[:, :, :], 2.0, v_[:, :, :],
                    op0=AL.mult, op1=AL.add)
                tcc = p2w.tile([128, 2, PB], dt.float32, tag=f"tcc{h}")
                nc.scalar.activation(tcc[:, :, :],
                                     cT[:, 2 * h:2 * h + 2, :], AF.Tanh)
                # hT stores h/16 (compensates the 16x fp8 Whh scaling)
                nc.vector.scalar_tensor_tensor(
                    (hTa if h == 0 else hTb)[:, :, t, :], sio[:, 2, :, :],
                    1.0 / 16.0, tcc[:, :, :], op0=AL.mult, op1=AL.mult)
            # filler: next slab window's XWT tile; P3 block-0 units once
            # their h block is complete
            if t < Ts - SLAB:
                xwt_unit(t // SLAB + 1, t % SLAB)
            if t >= TB + 2 and fillq:
                fillq.pop(0)()

        while fillq:
            fillq.pop(0)()
        for blk in range(1, NBLK):
            p3_block_tail(blk)

    nc.compile()
    return nc


def _prep_core(inputs, core, Ts=T):
    bf = ml_dtypes.bfloat16
    f8 = ml_dtypes.float8_e3m4
    s = slice(core * PB, (core + 1) * PB)
    ce = inputs["char_encoding"][s]
    teg = inputs["tag_encoding"][s]
    tos = inputs["true_output_seq"][s][:, :Ts]
    xs = np.concatenate(
        [np.zeros((PB, 1, NCH), np.float32), tos[:, 1:, :]], axis=1)
    # Whh/Wih rows: torch gate order (i,f,g,o) -> (i,f,o,g); feature chunk
    # ec split as (half, e2); tile j = half*8 + gt*2 + e2.
    # g-gate rows x2: tanh(g) computed as 2*sigmoid(2g) - 1 on device
    W = inputs["lstm_Whh"].reshape(4, 4, 128, E)[[0, 1, 3, 2]].copy()
    W[3] *= 2.0
    whhP = W.reshape(4, 2, 2, 128, E).transpose(4, 1, 0, 2, 3).reshape(E, 16, 128)
    V = inputs["lstm_Wih"].reshape(4, 4, 128, NCH)[[0, 1, 3, 2]].copy()
    V[3] *= 2.0
    wihP = V.reshape(4, 2, 2, 128, NCH).transpose(4, 1, 0, 2, 3).reshape(NCH, 16, 128)
    gbv = (inputs["lstm_bih"] + inputs["lstm_bhh"]).reshape(4, 4, 128)[[0, 1, 3, 2]].copy()
    gbv[3] *= 2.0
    gbias = gbv.reshape(4, 2, 2, 128).transpose(1, 0, 2, 3).reshape(16, 128)
    m = {
        "ceT": np.ascontiguousarray(ce.transpose(0, 2, 1)).astype(bf),
        "teT": np.ascontiguousarray(teg.transpose(0, 2, 1)).astype(bf),
        "xT": np.ascontiguousarray(xs.transpose(2, 1, 0)).astype(bf),
        "whhP": np.ascontiguousarray(whhP * 16.0).astype(f8),
        "whhB": np.ascontiguousarray(whhP).astype(bf),
        "wihP": np.ascontiguousarray(wihP).astype(bf),
        "gbias": np.ascontiguousarray(gbias).astype(np.float32),
        "wqc": inputs["ca_Wq"].astype(bf),
        "wkc": inputs["ca_Wk"].astype(bf),
        "wvc": inputs["ca_Wv"].astype(bf),
        "wocT": np.ascontiguousarray(inputs["ca_Wo"].T).astype(bf),
        "wqt": inputs["ta_Wq"].astype(bf),
        "wkt": inputs["ta_Wk"].astype(bf),
        "wvt": inputs["ta_Wv"].astype(bf),
        "wotT": np.ascontiguousarray(inputs["ta_Wo"].T).astype(bf),
        "bqc_col": inputs["ca_bq"][:, None].astype(bf),
        "bvc_col": inputs["ca_bv"][:, None].astype(bf),
        "bqt_col": inputs["ta_bq"][:, None].astype(bf),
        "bvt_col": inputs["ta_bv"][:, None].astype(bf),
        "boc": inputs["ca_bo"].astype(np.float32),
        "bot": inputs["ta_bo"].astype(np.float32),
        "outWT": np.ascontiguousarray(inputs["out_W"].T).astype(bf),
        "outb": inputs["out_b"].astype(np.float32),
        "h0T": np.ascontiguousarray(
            np.concatenate([inputs["char_hn"][0][s],
                            inputs["char_hn"][1][s]], -1).T).astype(bf),
        "c0T": np.ascontiguousarray(
            np.concatenate([inputs["char_cn"][0][s],
                            inputs["char_cn"][1][s]], -1).T).astype(np.float32),
    }
    return m


def kernel(**inputs):
    from concourse.bass_utils import run_bass_kernel_spmd

    inputs = {k: np.asarray(v, dtype=np.float32) for k, v in inputs.items()}
    if "nc" not in _cache:
        _cache["nc"] = _build(T)
    nc = _cache["nc"]
    in_maps = [_prep_core(inputs, c) for c in range(NCORES)]
    res = run_bass_kernel_spmd(nc, in_maps, list(range(NCORES)))
    _cache["last_res"] = res
    outs = [np.asarray(res.results[c]["out"]).transpose(0, 2, 1)
            for c in range(NCORES)]
    return np.ascontiguousarray(np.concatenate(outs, axis=0)).astype(np.float32)
